# revision 30
# baseline (speedup 1.0000x reference)
"""Trainium2 Bass kernel for nn_AttnApproximator (GQA attention + RoPE +
per-head shift correction), sharded over 8 NeuronCores.

Sharding: tensor-parallel over heads (4 groups of 8 query heads / 2 KV
heads) x data-parallel over batch (B=2) -> 8 cores. Each core computes a
partial output contribution [S, Dm] (its heads' slice of the attn @ Wo
contraction); the host sums the 4 head-group partials per batch element.

Per-core pipeline (everything stays transposed so no on-chip transposes
are needed):
  phase A: kT = (hs @ Wk).T with RoPE, v = hs @ Wv          (per s-slice)
  phase B: per s-slice of 512: qT = (hs @ Wq).T with RoPE, then causal
           attention per head in scores-transposed form:
             scoresT[sk, sq] = kT_tile.T @ qT    (f32r matmuls)
             expT = exp(scale * scoresT)          (ACT, bf16 out)
             attnT[dv, sq] += v_tile.T @ expT     (bf16 matmuls, PSUM acc)
             sums[1, sq]  += ones.T @ expT
           then delta = Ws_q.T-style per-head correction, normalize by
           broadcast(1/sums) via a K=1 PE matmul, add delta -> attnT(bf16)
  phase C: out[s, dm] = attnT.T @ Wo              (bf16 matmuls)
"""

import math
import numpy as np
import ml_dtypes

import bass_rust
import concourse.bass as bass
import concourse.tile as tile
from concourse import mybir
from concourse import bass_utils
from concourse.vector_clock import ScopedClock
from contextlib import ExitStack

# ---------------------------------------------------------------- constants
B, S, DM = 2, 2048, 4096
H, KV, D = 32, 8, 128
N_CORES = 8
TP = 4                    # head groups
HQ = H // TP              # 8 q heads per core
HKV = KV // TP            # 2 kv heads per core
GROUPS = H // KV          # 4
THETA = 10000.0
SQ = 512                  # s-slice width
NSL = S // SQ             # 4 slices
NK = DM // 128            # 32 contraction tiles
SCALE = 1.0 / math.sqrt(D)

F32 = mybir.dt.float32
F32R = mybir.dt.float32r
BF16 = mybir.dt.bfloat16
BF_NP = ml_dtypes.bfloat16


# ------------------------------------------------- walrus drain-wait fixup
class SplitDrainTileContext(tile.TileContext):
    """This container's walrus rejects >1 sync wait on the SP tail-drain
    CTRL instruction; split the gathered waits onto chained SP nops."""

    MAX_WAITS = 1

    def _drain_and_barrier(self, tick_clock, wait_clock):
        nc = self.nc
        drain_inst = nc.sync.drain()
        wait_clock.add_sem_waits(
            drain_inst.ins, ScopedClock({None: tick_clock.global_clock})
        )
        si = drain_inst.ins.sync_info
        waits = list(si.on_wait) if si is not None else []
        mw = self.MAX_WAITS
        if len(waits) > mw:
            drain_inst.ins.sync_info = bass_rust.SyncInfo(
                on_wait=waits[:mw], on_update=list(si.on_update)
            )
            for k in range(mw, len(waits), mw):
                nop = nc.sync.nop(nofuse=True, hint="drain_wait_split")
                nop.ins.sync_info = bass_rust.SyncInfo(
                    on_wait=waits[k : k + mw], on_update=[]
                )
        nc.all_engine_barrier()
        assert self.sems is not None
        popped = nc._tile_sem_poison_stack.pop()
        assert popped is self._sem_poison
        nc.clear_and_free_semaphores(list(self.sems.allocated().values()))
        nc.all_engine_barrier()


def _split_excess_waits(nc):
    """This walrus accepts 1 sync wait per instruction (2 for
    EventSemaphore). Tile emits more; move the excess onto same-engine
    NoOp carriers inserted immediately before the over-limit instruction."""
    uid = 0
    for fn in nc.m.functions:
        for bb in fn.blocks:
            new, changed = [], False
            for inst in bb.instructions:
                si = inst.sync_info
                waits = list(si.on_wait) if si is not None else []
                cap = 2 if inst.opcode == "EventSemaphore" else 1
                if len(waits) > cap:
                    changed = True
                    for w in waits[:-cap]:
                        nop = mybir.InstNoOp(
                            name=f"I-wsplit-{uid}",
                            engine=inst.engine,
                            bass_nofuse=True,
                            sync_info=mybir.SyncInfo(on_wait=[w], on_update=[]),
                        )
                        uid += 1
                        new.append(nop)
                    inst.sync_info = bass_rust.SyncInfo(
                        on_wait=waits[-cap:], on_update=list(si.on_update))
                new.append(inst)
            if changed:
                bb.instructions = new


# ---------------------------------------------------------------- builder
def _rope(nc, tmp_pool, out_ap, in_ps, cos_sl, sin_sl):
    """out = in*cosT + swap_halves(in)*sinT_signed ; in_ps is PSUM f32."""
    sw = tmp_pool.tile([128, SQ], F32, tag="rope_sw")
    nc.vector.tensor_copy(sw[0:64, :], in_ps[64:128, :])
    nc.vector.tensor_copy(sw[64:128, :], in_ps[0:64, :])
    nc.vector.tensor_mul(sw[:], sw[:], sin_sl)
    t2 = tmp_pool.tile([128, SQ], F32, tag="rope_t2")
    nc.vector.tensor_mul(t2[:], in_ps[:], cos_sl)
    nc.vector.tensor_add(out_ap, t2[:], sw[:])


def build_kernel():
    nc = bass.Bass("TRN2", target_bir_lowering=False, debug=False,
                   num_devices=N_CORES)

    # All inputs are pre-tiled on the host into the exact sbuf layouts so
    # every DMA is contiguous per partition (few large descriptors).
    din = lambda n, shp, dt: nc.dram_tensor(n, shp, dt, kind="ExternalInput").ap()
    hsT_t = din("hsT_t", [NSL, 128, NK, SQ], BF16)
    wq_t = din("wq_t", [HQ, 128, NK, D], BF16)
    wk_t = din("wk_t", [128, NK, HKV * D], BF16)
    wv_t = din("wv_t", [128, NK, HKV * D], BF16)
    wo_t = din("wo_t", [DM // 512, 128, HQ, 512], BF16)
    wsq = din("wsq", [128, HQ, D], BF16)
    wsk = din("wsk", [128, HQ, D], BF16)
    cosT = din("cosT", [D, S], F32)
    sinsg = din("sinsg", [D, S], F32)
    maskbig = din("maskbig", [D, 896], BF16)
    onesf_in = din("onesf", [1, 128], F32R)
    out = nc.dram_tensor("out", [S, DM], F32, kind="ExternalOutput").ap()

    with SplitDrainTileContext(nc) as tc, ExitStack() as octx:
        # ---------------- persistent sbuf ----------------
        pers = octx.enter_context(tc.tile_pool(name="pers", bufs=1))
        kT_sb = pers.tile([128, HKV, S], BF16, tag="kT")         # 8KB/p
        v_sb = pers.tile([128, S // 128, HKV * D], BF16, tag="v")  # 8KB/p
        attnT_sb = pers.tile([128, HQ, S], BF16, tag="attnT")    # 32KB/p
        cos_sb = pers.tile([128, S], F32, tag="cos")             # 8KB/p
        sin_sb = pers.tile([128, S], F32, tag="sin")             # 8KB/p
        mask_sb = pers.tile([128, 896], BF16, tag="mask")
        wsq_sb = pers.tile([128, HQ, D], BF16, tag="wsq")        # 2KB/p
        wsk_sb = pers.tile([128, HQ, D], BF16, tag="wsk")
        ones_bf = pers.tile([128, 1], BF16, tag="ones_bf")
        ones_f = pers.tile([1, 128], F32R, tag="ones_f")

        nc.sync.dma_start(cos_sb[:], cosT[:, :])
        nc.sync.dma_start(sin_sb[:], sinsg[:, :])
        nc.sync.dma_start(mask_sb[:], maskbig[:, :])
        nc.sync.dma_start(wsq_sb[:], wsq[:, :, :])
        nc.sync.dma_start(wsk_sb[:], wsk[:, :, :])
        nc.any.memset(ones_bf[:], 1.0)
        nc.sync.dma_start(ones_f[:], onesf_in[:, :])

        # ---------------- phase A: k/v projections ----------------
        with ExitStack() as actx:
            wkv_pool = actx.enter_context(tc.tile_pool(name="wkv", bufs=1))
            hstA_pool = actx.enter_context(tc.tile_pool(name="hstA", bufs=2))
            ptmpA = actx.enter_context(tc.tile_pool(name="ptmpA", bufs=3))
            psA = actx.enter_context(tc.tile_pool(name="psA", bufs=2, space="PSUM"))

            wk_sb = wkv_pool.tile([128, NK, HKV * D], BF16, tag="wk")  # 16KB/p
            wv_sb = wkv_pool.tile([128, NK, HKV * D], BF16, tag="wv")
            nc.sync.dma_start(wk_sb[:], wk_t[:, :, :])
            nc.sync.dma_start(wv_sb[:], wv_t[:, :, :])

            for n in range(NSL):
                sl = slice(n * SQ, (n + 1) * SQ)
                hst = hstA_pool.tile([128, NK, SQ], BF16, tag="hstA")
                nc.sync.dma_start(hst[:], hsT_t[n])
                # kT (transposed layout) + rope
                for kv in range(HKV):
                    ps = psA.tile([128, SQ], F32, tag="ps_k")
                    for k in range(NK):
                        nc.tensor.matmul(
                            ps[:], wk_sb[:, k, kv * D:(kv + 1) * D], hst[:, k, :],
                            start=(k == 0), stop=(k == NK - 1))
                    _rope(nc, ptmpA, kT_sb[:, kv, sl], ps,
                          cos_sb[:, sl], sin_sb[:, sl])
                # v natural layout [s, dv]
                for s4 in range(SQ // 128):
                    ps = psA.tile([128, HKV * D], F32, tag="ps_v")
                    for k in range(NK):
                        nc.tensor.matmul(
                            ps[:], hst[:, k, s4 * 128:(s4 + 1) * 128], wv_sb[:, k, :],
                            start=(k == 0), stop=(k == NK - 1))
                    nc.scalar.copy(v_sb[:, n * 4 + s4, :], ps[:])

        # ---------------- phase B: q proj + attention ----------------
        with ExitStack() as bctx:
            hstB_pool = bctx.enter_context(tc.tile_pool(name="hstB", bufs=1))
            wq_pool = bctx.enter_context(tc.tile_pool(name="wqp", bufs=2))
            qT_pool = bctx.enter_context(tc.tile_pool(name="qTp", bufs=2))
            exp_pool = bctx.enter_context(tc.tile_pool(name="expp", bufs=8))
            tmpB = bctx.enter_context(tc.tile_pool(name="tmpB", bufs=3))
            nrm_pool = bctx.enter_context(tc.tile_pool(name="nrmp", bufs=2))
            p_q = bctx.enter_context(tc.tile_pool(name="p_q", bufs=1, space="PSUM"))
            p_s = bctx.enter_context(tc.tile_pool(name="p_s", bufs=2, space="PSUM"))
            p_at = bctx.enter_context(tc.tile_pool(name="p_at", bufs=2, space="PSUM"))
            p_sm = bctx.enter_context(tc.tile_pool(name="p_sm", bufs=2, space="PSUM"))
            p_dl = bctx.enter_context(tc.tile_pool(name="p_dl", bufs=1, space="PSUM"))

            for n in range(NSL):
                sl = slice(n * SQ, (n + 1) * SQ)
                hst = hstB_pool.tile([128, NK, SQ], BF16, tag="hstB")
                nc.sync.dma_start(hst[:], hsT_t[n])
                qT = qT_pool.tile([128, HQ, SQ], BF16, tag="qT")
                for h in range(HQ):
                    wqc = wq_pool.tile([128, NK, D], BF16, tag="wqc")
                    nc.sync.dma_start(wqc[:], wq_t[h])
                    ps = p_q.tile([128, SQ], F32, tag="ps_q")
                    for k in range(NK):
                        nc.tensor.matmul(ps[:], wqc[:, k, :], hst[:, k, :],
                                         start=(k == 0), stop=(k == NK - 1))
                    _rope(nc, tmpB, qT[:, h, :], ps, cos_sb[:, sl], sin_sb[:, sl])

                nblk = 4 * (n + 1)
                for h in range(HQ):
                    kv = h // GROUPS
                    qh_r = qT[:, h, :]
                    ps_at = p_at.tile([128, SQ], F32, tag="ps_at")
                    ps_sm = p_sm.tile([1, SQ], F32, tag="ps_sm")
                    for t in range(nblk):
                        ps_sc = p_s.tile([128, SQ], F32, tag="ps_sc")
                        nc.tensor.matmul(
                            ps_sc[:],
                            kT_sb[:, kv, t * 128:(t + 1) * 128],
                            qh_r, start=True, stop=True)
                        ex = exp_pool.tile([128, SQ], BF16, tag="ex")
                        nc.scalar.activation(ex[:], ps_sc[:],
                                             mybir.ActivationFunctionType.Exp,
                                             scale=SCALE)
                        if t >= 4 * n:
                            r = t - 4 * n
                            exm = exp_pool.tile([128, SQ], BF16, tag="exm")
                            nc.vector.tensor_mul(
                                exm[:], ex[:], mask_sb[:, 384 - 128 * r: 896 - 128 * r])
                            ex = exm
                        nc.tensor.matmul(ps_at[:], v_sb[:, t, kv * D:(kv + 1) * D],
                                         ex[:], start=(t == 0), stop=(t == nblk - 1))
                        nc.tensor.matmul(ps_sm[:], ones_bf[:], ex[:],
                                         start=(t == 0), stop=(t == nblk - 1))
                    # delta: per-head shift correction
                    ps_dl = p_dl.tile([128, SQ], F32, tag="ps_dl")
                    nc.tensor.matmul(ps_dl[:], wsq_sb[:, h, :],
                                     qh_r, start=True, stop=False)
                    nc.tensor.matmul(ps_dl[:], wsk_sb[:, h, :],
                                     kT_sb[:, kv, sl],
                                     start=False, stop=True)
                    # normalize: attnT/sums + delta
                    rc = nrm_pool.tile([1, SQ], F32R, tag="rc")
                    with nc.allow_low_precision(reason="f32r recip for bcast"):
                        nc.vector.reciprocal(rc[:], ps_sm[:])
                    ps_b = p_s.tile([128, SQ], F32, tag="ps_sc")
                    nc.tensor.matmul(ps_b[:], ones_f[:], rc[:],
                                     start=True, stop=True)
                    bc_sb = tmpB.tile([128, SQ], F32, tag="bc_sb")
                    nc.vector.tensor_copy(bc_sb[:], ps_b[:])
                    t1 = tmpB.tile([128, SQ], F32, tag="t1")
                    nc.vector.tensor_mul(t1[:], ps_at[:], bc_sb[:])
                    nc.vector.tensor_add(attnT_sb[:, h, sl], t1[:], ps_dl[:])

        # ---------------- phase C: output projection ----------------
        with ExitStack() as cctx:
            wo_pool = cctx.enter_context(tc.tile_pool(name="wop", bufs=2))
            o_pool = cctx.enter_context(tc.tile_pool(name="op", bufs=4))
            p_o = cctx.enter_context(tc.tile_pool(name="p_o", bufs=4, space="PSUM"))
            for j in range(DM // 512):
                wo_sb = wo_pool.tile([128, HQ, 512], BF16, tag="wo")
                nc.sync.dma_start(wo_sb[:], wo_t[j])
                for m in range(S // 128):
                    ps = p_o.tile([128, 512], F32, tag="ps_o")
                    for t2 in range(HQ):
                        nc.tensor.matmul(ps[:], attnT_sb[:, t2, m * 128:(m + 1) * 128],
                                         wo_sb[:, t2, :],
                                         start=(t2 == 0), stop=(t2 == HQ - 1))
                    ot = o_pool.tile([128, 512], F32, tag="ot")
                    nc.scalar.copy(ot[:], ps[:])
                    nc.sync.dma_start(out[m * 128:(m + 1) * 128,
                                          j * 512:(j + 1) * 512], ot[:])
    _split_excess_waits(nc)
    return nc


# ---------------------------------------------------------------- host side
_CACHE = {}


def _prep_core_inputs(inputs, core):
    b, g = core // TP, core % TP
    hs = np.asarray(inputs["hidden_states"])[b]          # [S, DM] f32
    pos = np.asarray(inputs["position_ids"])[b]          # [S] int32
    Wq, Wk, Wv, Wo = (np.asarray(inputs[k]) for k in ("Wq", "Wk", "Wv", "Wo"))
    Ws_q, Ws_k = np.asarray(inputs["Ws_q"]), np.asarray(inputs["Ws_k"])

    qh0 = g * HQ                 # first global q head
    kv0 = g * HKV                # first global kv head

    inv_freq = 1.0 / (THETA ** (np.arange(0, D, 2, dtype=np.float64) / D))
    freqs = pos.astype(np.float64)[:, None] * inv_freq[None, :]   # [S, 64]
    cos = np.cos(freqs).astype(np.float32)
    sin = np.sin(freqs).astype(np.float32)
    cosT = np.ascontiguousarray(np.concatenate([cos, cos], axis=1).T)   # [128,S]
    sinsg = np.ascontiguousarray(np.concatenate([-sin, sin], axis=1).T)

    ii = np.arange(128)[:, None]
    cc = np.arange(896)[None, :]
    maskbig = ((cc - 384) >= ii).astype(BF_NP)

    # pre-tile into exact on-chip layouts (contiguous per-partition DMAs)
    hsT = hs.T.astype(BF_NP)                                   # [DM, S]
    hsT_t = np.ascontiguousarray(
        hsT.reshape(NK, 128, NSL, SQ).transpose(2, 1, 0, 3))   # [n, p, k, s]
    wq_c = Wq[:, qh0 * D:(qh0 + HQ) * D].astype(BF_NP)         # [DM, 1024]
    wq_t = np.ascontiguousarray(
        wq_c.reshape(NK, 128, HQ, D).transpose(2, 1, 0, 3))    # [h, p, k, m]
    wk_c = Wk[:, kv0 * D:(kv0 + HKV) * D].astype(BF_NP)
    wk_t = np.ascontiguousarray(
        wk_c.reshape(NK, 128, HKV * D).transpose(1, 0, 2))     # [p, k, m]
    wv_c = Wv[:, kv0 * D:(kv0 + HKV) * D].astype(BF_NP)
    wv_t = np.ascontiguousarray(
        wv_c.reshape(NK, 128, HKV * D).transpose(1, 0, 2))
    wo_c = Wo[qh0 * D:(qh0 + HQ) * D, :].astype(BF_NP)         # [1024, DM]
    wo_t = np.ascontiguousarray(
        wo_c.reshape(HQ, 128, DM // 512, 512).transpose(2, 1, 0, 3))  # [j,p,t,m]
    wsq_t = np.ascontiguousarray(
        Ws_q[qh0:qh0 + HQ].transpose(1, 0, 2)).astype(np.float32)  # [d, h, e]
    wsk_t = np.ascontiguousarray(
        Ws_k[qh0:qh0 + HQ].transpose(1, 0, 2)).astype(np.float32)
    return {
        "hsT_t": hsT_t,
        "wq_t": wq_t,
        "wk_t": wk_t,
        "wv_t": wv_t,
        "wo_t": wo_t,
        "wsq": wsq_t.astype(BF_NP),
        "wsk": wsk_t.astype(BF_NP),
        "cosT": cosT,
        "sinsg": sinsg,
        "maskbig": maskbig,
        "onesf": np.ones((1, 128), dtype=np.float32),
    }


def run(inputs, trace=False):
    if "nc" not in _CACHE:
        _CACHE["nc"] = build_kernel()
    nc = _CACHE["nc"]
    in_maps = [_prep_core_inputs(inputs, c) for c in range(N_CORES)]
    res = bass_utils.run_bass_kernel_spmd(
        nc, in_maps, core_ids=list(range(N_CORES)), trace=trace)
    full = np.zeros((B, S, DM), dtype=np.float32)
    for c in range(N_CORES):
        full[c // TP] += res.results[c]["out"]
    return full, res


def kernel(**inputs) -> np.ndarray:
    full, _ = run(inputs, trace=False)
    return full


# revision 37
# speedup vs baseline: 1.0184x; 1.0184x over previous
"""Trainium2 Bass kernel for nn_AttnApproximator (GQA attention + RoPE +
per-head shift correction), sharded over 8 NeuronCores.

Sharding: tensor-parallel over heads (4 groups of 8 query heads / 2 KV
heads) x data-parallel over batch (B=2) -> 8 cores. Each core computes a
partial output contribution [S, Dm] (its heads' slice of the attn @ Wo
contraction); the host sums the 4 head-group partials per batch element.

Per-core pipeline (everything stays transposed so no on-chip transposes
are needed):
  phase A: kT = (hs @ Wk).T with RoPE, v = hs @ Wv          (per s-slice)
  phase B: per s-slice of 512: qT = (hs @ Wq).T with RoPE, then causal
           attention per head in scores-transposed form:
             scoresT[sk, sq] = kT_tile.T @ qT    (f32r matmuls)
             expT = exp(scale * scoresT)          (ACT, bf16 out)
             attnT[dv, sq] += v_tile.T @ expT     (bf16 matmuls, PSUM acc)
             sums[1, sq]  += ones.T @ expT
           then delta = Ws_q.T-style per-head correction, normalize by
           broadcast(1/sums) via a K=1 PE matmul, add delta -> attnT(bf16)
  phase C: out[s, dm] = attnT.T @ Wo              (bf16 matmuls)
"""

import math
import numpy as np
import ml_dtypes

import bass_rust
import concourse.bass as bass
import concourse.tile as tile
from concourse import mybir
from concourse import bass_utils
from concourse.vector_clock import ScopedClock
from contextlib import ExitStack

# ---------------------------------------------------------------- constants
B, S, DM = 2, 2048, 4096
H, KV, D = 32, 8, 128
N_CORES = 8
TP = 4                    # head groups
HQ = H // TP              # 8 q heads per core
HKV = KV // TP            # 2 kv heads per core
GROUPS = H // KV          # 4
THETA = 10000.0
SQ = 512                  # s-slice width
NSL = S // SQ             # 4 slices
NK = DM // 128            # 32 contraction tiles
SCALE = 1.0 / math.sqrt(D)

F32 = mybir.dt.float32
F32R = mybir.dt.float32r
BF16 = mybir.dt.bfloat16
BF_NP = ml_dtypes.bfloat16


# ------------------------------------------------- walrus drain-wait fixup
class SplitDrainTileContext(tile.TileContext):
    """This container's walrus rejects >1 sync wait on the SP tail-drain
    CTRL instruction; split the gathered waits onto chained SP nops."""

    MAX_WAITS = 1

    def _drain_and_barrier(self, tick_clock, wait_clock):
        nc = self.nc
        drain_inst = nc.sync.drain()
        wait_clock.add_sem_waits(
            drain_inst.ins, ScopedClock({None: tick_clock.global_clock})
        )
        si = drain_inst.ins.sync_info
        waits = list(si.on_wait) if si is not None else []
        mw = self.MAX_WAITS
        if len(waits) > mw:
            drain_inst.ins.sync_info = bass_rust.SyncInfo(
                on_wait=waits[:mw], on_update=list(si.on_update)
            )
            for k in range(mw, len(waits), mw):
                nop = nc.sync.nop(nofuse=True, hint="drain_wait_split")
                nop.ins.sync_info = bass_rust.SyncInfo(
                    on_wait=waits[k : k + mw], on_update=[]
                )
        nc.all_engine_barrier()
        assert self.sems is not None
        popped = nc._tile_sem_poison_stack.pop()
        assert popped is self._sem_poison
        nc.clear_and_free_semaphores(list(self.sems.allocated().values()))
        nc.all_engine_barrier()


def _split_excess_waits(nc):
    """This walrus accepts 1 sync wait per instruction (2 for
    EventSemaphore). Tile emits more; move the excess onto same-engine
    NoOp carriers inserted immediately before the over-limit instruction."""
    uid = 0
    for fn in nc.m.functions:
        for bb in fn.blocks:
            new, changed = [], False
            for inst in bb.instructions:
                si = inst.sync_info
                waits = list(si.on_wait) if si is not None else []
                cap = 2 if inst.opcode == "EventSemaphore" else 1
                if len(waits) > cap:
                    changed = True
                    for w in waits[:-cap]:
                        nop = mybir.InstNoOp(
                            name=f"I-wsplit-{uid}",
                            engine=inst.engine,
                            bass_nofuse=True,
                            sync_info=mybir.SyncInfo(on_wait=[w], on_update=[]),
                        )
                        uid += 1
                        new.append(nop)
                    inst.sync_info = bass_rust.SyncInfo(
                        on_wait=waits[-cap:], on_update=list(si.on_update))
                new.append(inst)
            if changed:
                bb.instructions = new


# ---------------------------------------------------------------- builder
def _rope(nc, tmp_pool, out_ap, in_ps, cos_sl, sin_sl):
    """out = in*cosT + swap_halves(in)*sinT_signed ; in_ps is PSUM f32.
    First op drains PSUM via ACT so the bank frees fast (p_q bufs=1)."""
    q_sb = tmp_pool.tile([128, SQ], F32, tag="rope_q")
    nc.scalar.copy(q_sb[:], in_ps[:])
    sw = tmp_pool.tile([128, SQ], F32, tag="rope_sw")
    nc.vector.tensor_copy(sw[0:64, :], q_sb[64:128, :])
    nc.vector.tensor_copy(sw[64:128, :], q_sb[0:64, :])
    nc.vector.tensor_mul(sw[:], sw[:], sin_sl)
    t2 = tmp_pool.tile([128, SQ], F32, tag="rope_t2")
    nc.vector.tensor_mul(t2[:], q_sb[:], cos_sl)
    nc.vector.tensor_add(out_ap, t2[:], sw[:])


def build_kernel():
    nc = bass.Bass("TRN2", target_bir_lowering=False, debug=False,
                   num_devices=N_CORES)

    # All inputs are pre-tiled on the host into the exact sbuf layouts so
    # every DMA is contiguous per partition (few large descriptors).
    din = lambda n, shp, dt: nc.dram_tensor(n, shp, dt, kind="ExternalInput").ap()
    hsT_t = din("hsT_t", [NSL, 128, NK, SQ], BF16)
    wq_t = din("wq_t", [HQ, 128, NK, D], BF16)
    wk_t = din("wk_t", [128, NK, HKV * D], BF16)
    wv_t = din("wv_t", [128, NK, HKV * D], BF16)
    wo_t = din("wo_t", [DM // 512, 128, HQ, 512], BF16)
    wsq = din("wsq", [128, HQ, D], BF16)
    wsk = din("wsk", [128, HQ, D], BF16)
    cosT = din("cosT", [D, S], F32)
    sinsg = din("sinsg", [D, S], F32)
    maskbig = din("maskbig", [D, 896], BF16)
    onesf_in = din("onesf", [1, 128], mybir.dt.float16)
    out = nc.dram_tensor("out", [S, DM], F32, kind="ExternalOutput").ap()

    with SplitDrainTileContext(nc) as tc, ExitStack() as octx:
        # ---------------- persistent sbuf ----------------
        pers = octx.enter_context(tc.tile_pool(name="pers", bufs=1))
        kT_sb = pers.tile([128, HKV, S], BF16, tag="kT")         # 8KB/p
        v_sb = pers.tile([128, S // 128, HKV * D], BF16, tag="v")  # 8KB/p
        attnT_sb = pers.tile([128, HQ, S], BF16, tag="attnT")    # 32KB/p
        cos_sb = pers.tile([128, S], F32, tag="cos")             # 8KB/p
        sin_sb = pers.tile([128, S], F32, tag="sin")             # 8KB/p
        mask_sb = pers.tile([128, 896], BF16, tag="mask")
        wsq_sb = pers.tile([128, HQ, D], BF16, tag="wsq")        # 2KB/p
        wsk_sb = pers.tile([128, HQ, D], BF16, tag="wsk")
        ones_bf = pers.tile([128, 1], BF16, tag="ones_bf")
        ones_f = pers.tile([1, 128], mybir.dt.float16, tag="ones_f")

        nc.sync.dma_start(cos_sb[:], cosT[:, :])
        nc.sync.dma_start(sin_sb[:], sinsg[:, :])
        nc.sync.dma_start(mask_sb[:], maskbig[:, :])
        nc.sync.dma_start(wsq_sb[:], wsq[:, :, :])
        nc.sync.dma_start(wsk_sb[:], wsk[:, :, :])
        nc.any.memset(ones_bf[:], 1.0)
        nc.sync.dma_start(ones_f[:], onesf_in[:, :])

        # ---------------- phase A: k/v projections ----------------
        with ExitStack() as actx:
            wkv_pool = actx.enter_context(tc.tile_pool(name="wkv", bufs=1))
            hstA_pool = actx.enter_context(tc.tile_pool(name="hstA", bufs=2))
            ptmpA = actx.enter_context(tc.tile_pool(name="ptmpA", bufs=3))
            psA = actx.enter_context(tc.tile_pool(name="psA", bufs=2, space="PSUM"))

            wk_sb = wkv_pool.tile([128, NK, HKV * D], BF16, tag="wk")  # 16KB/p
            wv_sb = wkv_pool.tile([128, NK, HKV * D], BF16, tag="wv")
            nc.sync.dma_start(wk_sb[:], wk_t[:, :, :])
            nc.sync.dma_start(wv_sb[:], wv_t[:, :, :])

            for n in range(NSL):
                sl = slice(n * SQ, (n + 1) * SQ)
                hst = hstA_pool.tile([128, NK, SQ], BF16, tag="hstA")
                nc.sync.dma_start(hst[:], hsT_t[n])
                # kT (transposed layout) + rope
                for kv in range(HKV):
                    ps = psA.tile([128, SQ], F32, tag="ps_k")
                    for k in range(NK):
                        nc.tensor.matmul(
                            ps[:], wk_sb[:, k, kv * D:(kv + 1) * D], hst[:, k, :],
                            start=(k == 0), stop=(k == NK - 1))
                    _rope(nc, ptmpA, kT_sb[:, kv, sl], ps,
                          cos_sb[:, sl], sin_sb[:, sl])
                # v natural layout [s, dv]
                for s4 in range(SQ // 128):
                    ps = psA.tile([128, HKV * D], F32, tag="ps_v")
                    for k in range(NK):
                        nc.tensor.matmul(
                            ps[:], hst[:, k, s4 * 128:(s4 + 1) * 128], wv_sb[:, k, :],
                            start=(k == 0), stop=(k == NK - 1))
                    nc.scalar.copy(v_sb[:, n * 4 + s4, :], ps[:])

        # ---------------- phase B: q proj + attention ----------------
        with ExitStack() as bctx:
            hstB_pool = bctx.enter_context(tc.tile_pool(name="hstB", bufs=1))
            wq_pool = bctx.enter_context(tc.tile_pool(name="wqp", bufs=2))
            qT_pool = bctx.enter_context(tc.tile_pool(name="qTp", bufs=2))
            exp_pool = bctx.enter_context(tc.tile_pool(name="expp", bufs=8))
            tmpB = bctx.enter_context(tc.tile_pool(name="tmpB", bufs=3))
            nrm_pool = bctx.enter_context(tc.tile_pool(name="nrmp", bufs=2))
            p_q = bctx.enter_context(tc.tile_pool(name="p_q", bufs=1, space="PSUM"))
            p_s = bctx.enter_context(tc.tile_pool(name="p_s", bufs=2, space="PSUM"))
            p_at = bctx.enter_context(tc.tile_pool(name="p_at", bufs=2, space="PSUM"))
            p_sm = bctx.enter_context(tc.tile_pool(name="p_sm", bufs=1, space="PSUM"))
            p_dl = bctx.enter_context(tc.tile_pool(name="p_dl", bufs=1, space="PSUM"))
            p_bc = bctx.enter_context(tc.tile_pool(name="p_bc", bufs=1, space="PSUM"))

            for n in range(NSL):
                sl = slice(n * SQ, (n + 1) * SQ)
                hst = hstB_pool.tile([128, NK, SQ], BF16, tag="hstB")
                nc.sync.dma_start(hst[:], hsT_t[n])
                qT = qT_pool.tile([128, HQ, SQ], BF16, tag="qT")
                for h in range(HQ):
                    wqc = wq_pool.tile([128, NK, D], BF16, tag="wqc")
                    nc.sync.dma_start(wqc[:], wq_t[h])
                    ps = p_q.tile([128, SQ], F32, tag="ps_q")
                    for k in range(NK):
                        nc.tensor.matmul(ps[:], wqc[:, k, :], hst[:, k, :],
                                         start=(k == 0), stop=(k == NK - 1))
                    _rope(nc, tmpB, qT[:, h, :], ps, cos_sb[:, sl], sin_sb[:, sl])

                nblk = 4 * (n + 1)
                for h in range(HQ):
                    kv = h // GROUPS
                    qh_r = qT[:, h, :]
                    ps_at = p_at.tile([128, SQ], F32, tag="ps_at")
                    ps_sm = p_sm.tile([1, SQ], F32, tag="ps_sm")
                    for t in range(nblk):
                        ps_sc = p_s.tile([128, SQ], F32, tag="ps_sc")
                        nc.tensor.matmul(
                            ps_sc[:],
                            kT_sb[:, kv, t * 128:(t + 1) * 128],
                            qh_r, start=True, stop=True)
                        ex = exp_pool.tile([128, SQ], BF16, tag="ex")
                        nc.scalar.activation(ex[:], ps_sc[:],
                                             mybir.ActivationFunctionType.Exp,
                                             scale=SCALE)
                        if t >= 4 * n:
                            r = t - 4 * n
                            exm = exp_pool.tile([128, SQ], BF16, tag="exm")
                            nc.vector.tensor_mul(
                                exm[:], ex[:], mask_sb[:, 384 - 128 * r: 896 - 128 * r])
                            ex = exm
                        nc.tensor.matmul(ps_at[:], v_sb[:, t, kv * D:(kv + 1) * D],
                                         ex[:], start=(t == 0), stop=(t == nblk - 1))
                        nc.tensor.matmul(ps_sm[:], ones_bf[:], ex[:],
                                         start=(t == 0), stop=(t == nblk - 1))
                    # delta: per-head shift correction
                    ps_dl = p_dl.tile([128, SQ], F32, tag="ps_dl")
                    nc.tensor.matmul(ps_dl[:], wsq_sb[:, h, :],
                                     qh_r, start=True, stop=False)
                    nc.tensor.matmul(ps_dl[:], wsk_sb[:, h, :],
                                     kT_sb[:, kv, sl],
                                     start=False, stop=True)
                    # normalize: attnT/sums + delta
                    rc = nrm_pool.tile([1, SQ], mybir.dt.float16, tag="rc")
                    with nc.allow_low_precision(reason="fp16 recip for bcast"):
                        nc.vector.reciprocal(rc[:], ps_sm[:])
                    ps_b = p_bc.tile([128, SQ], F32, tag="ps_b")
                    nc.tensor.matmul(ps_b[:], ones_f[:], rc[:],
                                     start=True, stop=True)
                    bc_sb = tmpB.tile([128, SQ], F32, tag="bc_sb")
                    nc.scalar.copy(bc_sb[:], ps_b[:])
                    t1 = tmpB.tile([128, SQ], F32, tag="t1")
                    nc.vector.tensor_mul(t1[:], ps_at[:], bc_sb[:])
                    nc.vector.tensor_add(attnT_sb[:, h, sl], t1[:], ps_dl[:])

        # ---------------- phase C: output projection ----------------
        with ExitStack() as cctx:
            wo_pool = cctx.enter_context(tc.tile_pool(name="wop", bufs=2))
            o_pool = cctx.enter_context(tc.tile_pool(name="op", bufs=4))
            p_o = cctx.enter_context(tc.tile_pool(name="p_o", bufs=4, space="PSUM"))
            for j in range(DM // 512):
                wo_sb = wo_pool.tile([128, HQ, 512], BF16, tag="wo")
                nc.sync.dma_start(wo_sb[:], wo_t[j])
                for m in range(S // 128):
                    ps = p_o.tile([128, 512], F32, tag="ps_o")
                    for t2 in range(HQ):
                        nc.tensor.matmul(ps[:], attnT_sb[:, t2, m * 128:(m + 1) * 128],
                                         wo_sb[:, t2, :],
                                         start=(t2 == 0), stop=(t2 == HQ - 1))
                    ot = o_pool.tile([128, 512], F32, tag="ot")
                    nc.scalar.copy(ot[:], ps[:])
                    nc.sync.dma_start(out[m * 128:(m + 1) * 128,
                                          j * 512:(j + 1) * 512], ot[:])
    _split_excess_waits(nc)
    return nc


# ---------------------------------------------------------------- host side
_CACHE = {}


def _prep_core_inputs(inputs, core):
    b, g = core // TP, core % TP
    hs = np.asarray(inputs["hidden_states"])[b]          # [S, DM] f32
    pos = np.asarray(inputs["position_ids"])[b]          # [S] int32
    Wq, Wk, Wv, Wo = (np.asarray(inputs[k]) for k in ("Wq", "Wk", "Wv", "Wo"))
    Ws_q, Ws_k = np.asarray(inputs["Ws_q"]), np.asarray(inputs["Ws_k"])

    qh0 = g * HQ                 # first global q head
    kv0 = g * HKV                # first global kv head

    inv_freq = 1.0 / (THETA ** (np.arange(0, D, 2, dtype=np.float64) / D))
    freqs = pos.astype(np.float64)[:, None] * inv_freq[None, :]   # [S, 64]
    cos = np.cos(freqs).astype(np.float32)
    sin = np.sin(freqs).astype(np.float32)
    cosT = np.ascontiguousarray(np.concatenate([cos, cos], axis=1).T)   # [128,S]
    sinsg = np.ascontiguousarray(np.concatenate([-sin, sin], axis=1).T)

    ii = np.arange(128)[:, None]
    cc = np.arange(896)[None, :]
    maskbig = ((cc - 384) >= ii).astype(BF_NP)

    # pre-tile into exact on-chip layouts (contiguous per-partition DMAs)
    hsT = hs.T.astype(BF_NP)                                   # [DM, S]
    hsT_t = np.ascontiguousarray(
        hsT.reshape(NK, 128, NSL, SQ).transpose(2, 1, 0, 3))   # [n, p, k, s]
    wq_c = Wq[:, qh0 * D:(qh0 + HQ) * D].astype(BF_NP)         # [DM, 1024]
    wq_t = np.ascontiguousarray(
        wq_c.reshape(NK, 128, HQ, D).transpose(2, 1, 0, 3))    # [h, p, k, m]
    wk_c = Wk[:, kv0 * D:(kv0 + HKV) * D].astype(BF_NP)
    wk_t = np.ascontiguousarray(
        wk_c.reshape(NK, 128, HKV * D).transpose(1, 0, 2))     # [p, k, m]
    wv_c = Wv[:, kv0 * D:(kv0 + HKV) * D].astype(BF_NP)
    wv_t = np.ascontiguousarray(
        wv_c.reshape(NK, 128, HKV * D).transpose(1, 0, 2))
    wo_c = Wo[qh0 * D:(qh0 + HQ) * D, :].astype(BF_NP)         # [1024, DM]
    wo_t = np.ascontiguousarray(
        wo_c.reshape(HQ, 128, DM // 512, 512).transpose(2, 1, 0, 3))  # [j,p,t,m]
    wsq_t = np.ascontiguousarray(
        Ws_q[qh0:qh0 + HQ].transpose(1, 0, 2)).astype(np.float32)  # [d, h, e]
    wsk_t = np.ascontiguousarray(
        Ws_k[qh0:qh0 + HQ].transpose(1, 0, 2)).astype(np.float32)
    return {
        "hsT_t": hsT_t,
        "wq_t": wq_t,
        "wk_t": wk_t,
        "wv_t": wv_t,
        "wo_t": wo_t,
        "wsq": wsq_t.astype(BF_NP),
        "wsk": wsk_t.astype(BF_NP),
        "cosT": cosT,
        "sinsg": sinsg,
        "maskbig": maskbig,
        "onesf": np.ones((1, 128), dtype=np.float16),
    }


def run(inputs, trace=False):
    if "nc" not in _CACHE:
        _CACHE["nc"] = build_kernel()
    nc = _CACHE["nc"]
    in_maps = [_prep_core_inputs(inputs, c) for c in range(N_CORES)]
    res = bass_utils.run_bass_kernel_spmd(
        nc, in_maps, core_ids=list(range(N_CORES)), trace=trace)
    full = np.zeros((B, S, DM), dtype=np.float32)
    for c in range(N_CORES):
        full[c // TP] += res.results[c]["out"]
    return full, res


def kernel(**inputs) -> np.ndarray:
    full, _ = run(inputs, trace=False)
    return full


# revision 40
# speedup vs baseline: 1.0806x; 1.0610x over previous
"""Trainium2 Bass kernel for nn_AttnApproximator (GQA attention + RoPE +
per-head shift correction), sharded over 8 NeuronCores.

Sharding: tensor-parallel over heads (4 groups of 8 query heads / 2 KV
heads) x data-parallel over batch (B=2) -> 8 cores. Each core computes a
partial output contribution [S, Dm] (its heads' slice of the attn @ Wo
contraction); the host sums the 4 head-group partials per batch element.

Per-core pipeline (everything stays transposed so no on-chip transposes
are needed):
  phase A: kT = (hs @ Wk).T with RoPE, v = hs @ Wv          (per s-slice)
  phase B: per s-slice of 512: qT = (hs @ Wq).T with RoPE, then causal
           attention per head in scores-transposed form:
             scoresT[sk, sq] = kT_tile.T @ qT    (f32r matmuls)
             expT = exp(scale * scoresT)          (ACT, bf16 out)
             attnT[dv, sq] += v_tile.T @ expT     (bf16 matmuls, PSUM acc)
             sums[1, sq]  += ones.T @ expT
           then delta = Ws_q.T-style per-head correction, normalize by
           broadcast(1/sums) via a K=1 PE matmul, add delta -> attnT(bf16)
  phase C: out[s, dm] = attnT.T @ Wo              (bf16 matmuls)
"""

import math
import numpy as np
import ml_dtypes

import bass_rust
import concourse.bass as bass
import concourse.tile as tile
from concourse import mybir
from concourse import bass_utils
from concourse.vector_clock import ScopedClock
from contextlib import ExitStack

# ---------------------------------------------------------------- constants
B, S, DM = 2, 2048, 4096
H, KV, D = 32, 8, 128
N_CORES = 8
TP = 4                    # head groups
HQ = H // TP              # 8 q heads per core
HKV = KV // TP            # 2 kv heads per core
GROUPS = H // KV          # 4
THETA = 10000.0
SQ = 512                  # s-slice width
NSL = S // SQ             # 4 slices
NK = DM // 128            # 32 contraction tiles
SCALE = 1.0 / math.sqrt(D)

F32 = mybir.dt.float32
F32R = mybir.dt.float32r
BF16 = mybir.dt.bfloat16
BF_NP = ml_dtypes.bfloat16


# ------------------------------------------------- walrus drain-wait fixup
class SplitDrainTileContext(tile.TileContext):
    """This container's walrus rejects >1 sync wait on the SP tail-drain
    CTRL instruction; split the gathered waits onto chained SP nops."""

    MAX_WAITS = 1

    def _drain_and_barrier(self, tick_clock, wait_clock):
        nc = self.nc
        drain_inst = nc.sync.drain()
        wait_clock.add_sem_waits(
            drain_inst.ins, ScopedClock({None: tick_clock.global_clock})
        )
        si = drain_inst.ins.sync_info
        waits = list(si.on_wait) if si is not None else []
        mw = self.MAX_WAITS
        if len(waits) > mw:
            drain_inst.ins.sync_info = bass_rust.SyncInfo(
                on_wait=waits[:mw], on_update=list(si.on_update)
            )
            for k in range(mw, len(waits), mw):
                nop = nc.sync.nop(nofuse=True, hint="drain_wait_split")
                nop.ins.sync_info = bass_rust.SyncInfo(
                    on_wait=waits[k : k + mw], on_update=[]
                )
        nc.all_engine_barrier()
        assert self.sems is not None
        popped = nc._tile_sem_poison_stack.pop()
        assert popped is self._sem_poison
        nc.clear_and_free_semaphores(list(self.sems.allocated().values()))
        nc.all_engine_barrier()


def _split_excess_waits(nc):
    """This walrus accepts 1 sync wait per instruction (2 for
    EventSemaphore). Tile emits more; move the excess onto same-engine
    NoOp carriers inserted immediately before the over-limit instruction."""
    uid = 0
    for fn in nc.m.functions:
        for bb in fn.blocks:
            new, changed = [], False
            for inst in bb.instructions:
                si = inst.sync_info
                waits = list(si.on_wait) if si is not None else []
                cap = 2 if inst.opcode == "EventSemaphore" else 1
                if len(waits) > cap:
                    changed = True
                    for w in waits[:-cap]:
                        nop = mybir.InstNoOp(
                            name=f"I-wsplit-{uid}",
                            engine=inst.engine,
                            bass_nofuse=True,
                            sync_info=mybir.SyncInfo(on_wait=[w], on_update=[]),
                        )
                        uid += 1
                        new.append(nop)
                    inst.sync_info = bass_rust.SyncInfo(
                        on_wait=waits[-cap:], on_update=list(si.on_update))
                new.append(inst)
            if changed:
                bb.instructions = new


# ---------------------------------------------------------------- builder
def _rope(nc, tmp_pool, out_ap, in_ps, cos_sl, sin_sl):
    """out = in*cosT + swap_halves(in)*sinT_signed ; in_ps is PSUM f32.
    First op drains PSUM via ACT so the bank frees fast (p_q bufs=1)."""
    q_sb = tmp_pool.tile([128, SQ], F32, tag="rope_q")
    nc.scalar.copy(q_sb[:], in_ps[:])
    sw = tmp_pool.tile([128, SQ], F32, tag="rope_sw")
    nc.vector.tensor_copy(sw[0:64, :], q_sb[64:128, :])
    nc.vector.tensor_copy(sw[64:128, :], q_sb[0:64, :])
    nc.vector.tensor_mul(sw[:], sw[:], sin_sl)
    t2 = tmp_pool.tile([128, SQ], F32, tag="rope_t2")
    nc.vector.tensor_mul(t2[:], q_sb[:], cos_sl)
    nc.vector.tensor_add(out_ap, t2[:], sw[:])


def build_kernel():
    nc = bass.Bass("TRN2", target_bir_lowering=False, debug=False,
                   num_devices=N_CORES)

    # All inputs are pre-tiled on the host into the exact sbuf layouts so
    # every DMA is contiguous per partition (few large descriptors).
    din = lambda n, shp, dt: nc.dram_tensor(n, shp, dt, kind="ExternalInput").ap()
    hsT_t = din("hsT_t", [NSL, 128, NK, SQ], BF16)
    wq_t = din("wq_t", [HQ, 128, NK, D], BF16)
    wk_t = din("wk_t", [128, NK, HKV * D], BF16)
    wv_t = din("wv_t", [128, NK, HKV * D], BF16)
    wo_t = din("wo_t", [DM // 512, 128, HQ, 512], BF16)
    wsq = din("wsq", [128, HQ, D], BF16)
    wsk = din("wsk", [128, HQ, D], BF16)
    cosT = din("cosT", [D, S], F32)
    sinsg = din("sinsg", [D, S], F32)
    maskbig = din("maskbig", [D, 896], BF16)
    onesf_in = din("onesf", [1, 128], mybir.dt.float16)
    out = nc.dram_tensor("out", [S, DM], F32, kind="ExternalOutput").ap()

    with SplitDrainTileContext(nc) as tc, ExitStack() as octx:
        # ---------------- persistent sbuf ----------------
        pers = octx.enter_context(tc.tile_pool(name="pers", bufs=1))
        kT_sb = pers.tile([128, HKV, S], BF16, tag="kT")         # 8KB/p
        v_sb = pers.tile([128, S // 128, HKV * D], BF16, tag="v")  # 8KB/p
        attnT_sb = pers.tile([128, HQ, S], BF16, tag="attnT")    # 32KB/p
        cos_sb = pers.tile([128, S], F32, tag="cos")             # 8KB/p
        sin_sb = pers.tile([128, S], F32, tag="sin")             # 8KB/p
        mask_sb = pers.tile([128, 896], BF16, tag="mask")
        wsq_sb = pers.tile([128, HQ, D], BF16, tag="wsq")        # 2KB/p
        wsk_sb = pers.tile([128, HQ, D], BF16, tag="wsk")
        ones_bf = pers.tile([128, 1], BF16, tag="ones_bf")
        ones_f = pers.tile([1, 128], mybir.dt.float16, tag="ones_f")

        nc.any.memset(ones_bf[:], 1.0)
        nc.sync.dma_start(ones_f[:], onesf_in[:, :])

        # ---------------- phase A: k/v projections ----------------
        with ExitStack() as actx:
            wkv_pool = actx.enter_context(tc.tile_pool(name="wkv", bufs=1))
            hstA_pool = actx.enter_context(tc.tile_pool(name="hstA", bufs=2))
            ptmpA = actx.enter_context(tc.tile_pool(name="ptmpA", bufs=3))
            psA = actx.enter_context(tc.tile_pool(name="psA", bufs=2, space="PSUM"))

            wk_sb = wkv_pool.tile([128, NK, HKV * D], BF16, tag="wk")  # 16KB/p
            wv_sb = wkv_pool.tile([128, NK, HKV * D], BF16, tag="wv")
            nc.sync.dma_start(wk_sb[:], wk_t[:, :, :])
            nc.sync.dma_start(wv_sb[:], wv_t[:, :, :])

            for n in range(NSL):
                sl = slice(n * SQ, (n + 1) * SQ)
                hst = hstA_pool.tile([128, NK, SQ], BF16, tag="hstA")
                nc.sync.dma_start(hst[:], hsT_t[n])
                if n == 0:
                    # issue after the critical-path loads so they don't
                    # steal startup DMA bandwidth
                    nc.sync.dma_start(cos_sb[:], cosT[:, :])
                    nc.sync.dma_start(sin_sb[:], sinsg[:, :])
                    nc.sync.dma_start(mask_sb[:], maskbig[:, :])
                    nc.sync.dma_start(wsq_sb[:], wsq[:, :, :])
                    nc.sync.dma_start(wsk_sb[:], wsk[:, :, :])
                # kT (transposed layout) + rope
                for kv in range(HKV):
                    ps = psA.tile([128, SQ], F32, tag="ps_k")
                    for k in range(NK):
                        nc.tensor.matmul(
                            ps[:], wk_sb[:, k, kv * D:(kv + 1) * D], hst[:, k, :],
                            start=(k == 0), stop=(k == NK - 1))
                    _rope(nc, ptmpA, kT_sb[:, kv, sl], ps,
                          cos_sb[:, sl], sin_sb[:, sl])
                # v natural layout [s, dv]
                for s4 in range(SQ // 128):
                    ps = psA.tile([128, HKV * D], F32, tag="ps_v")
                    for k in range(NK):
                        nc.tensor.matmul(
                            ps[:], hst[:, k, s4 * 128:(s4 + 1) * 128], wv_sb[:, k, :],
                            start=(k == 0), stop=(k == NK - 1))
                    nc.scalar.copy(v_sb[:, n * 4 + s4, :], ps[:])

        # ---------------- phase B: q proj + attention ----------------
        with ExitStack() as bctx:
            hstB_pool = bctx.enter_context(tc.tile_pool(name="hstB", bufs=1))
            wq_pool = bctx.enter_context(tc.tile_pool(name="wqp", bufs=2))
            qT_pool = bctx.enter_context(tc.tile_pool(name="qTp", bufs=2))
            exp_pool = bctx.enter_context(tc.tile_pool(name="expp", bufs=16))
            tmpB = bctx.enter_context(tc.tile_pool(name="tmpB", bufs=3))
            nrm_pool = bctx.enter_context(tc.tile_pool(name="nrmp", bufs=2))
            p_q = bctx.enter_context(tc.tile_pool(name="p_q", bufs=1, space="PSUM"))
            p_s = bctx.enter_context(tc.tile_pool(name="p_s", bufs=2, space="PSUM"))
            p_at = bctx.enter_context(tc.tile_pool(name="p_at", bufs=2, space="PSUM"))
            p_sm = bctx.enter_context(tc.tile_pool(name="p_sm", bufs=1, space="PSUM"))
            p_dl = bctx.enter_context(tc.tile_pool(name="p_dl", bufs=1, space="PSUM"))
            p_bc = bctx.enter_context(tc.tile_pool(name="p_bc", bufs=1, space="PSUM"))

            for n in range(NSL):
                sl = slice(n * SQ, (n + 1) * SQ)
                hst = hstB_pool.tile([128, NK, SQ], BF16, tag="hstB")
                nc.sync.dma_start(hst[:], hsT_t[n])
                qT = qT_pool.tile([128, HQ, SQ], BF16, tag="qT")
                for h in range(HQ):
                    wqc = wq_pool.tile([128, NK, D], BF16, tag="wqc")
                    nc.sync.dma_start(wqc[:], wq_t[h])
                    ps = p_q.tile([128, SQ], F32, tag="ps_q")
                    for k in range(NK):
                        nc.tensor.matmul(ps[:], wqc[:, k, :], hst[:, k, :],
                                         start=(k == 0), stop=(k == NK - 1))
                    _rope(nc, tmpB, qT[:, h, :], ps, cos_sb[:, sl], sin_sb[:, sl])

                nblk = 4 * (n + 1)
                for h in range(HQ):
                    kv = h // GROUPS
                    qh_r = qT[:, h, :]
                    ps_at = p_at.tile([128, SQ], F32, tag="ps_at")
                    ps_sm = p_sm.tile([1, SQ], F32, tag="ps_sm")
                    for t in range(nblk):
                        ps_sc = p_s.tile([128, SQ], F32, tag="ps_sc")
                        nc.tensor.matmul(
                            ps_sc[:],
                            kT_sb[:, kv, t * 128:(t + 1) * 128],
                            qh_r, start=True, stop=True)
                        ex = exp_pool.tile([128, SQ], BF16, tag="ex")
                        nc.scalar.activation(ex[:], ps_sc[:],
                                             mybir.ActivationFunctionType.Exp,
                                             scale=SCALE)
                        if t >= 4 * n:
                            r = t - 4 * n
                            exm = exp_pool.tile([128, SQ], BF16, tag="exm")
                            nc.vector.tensor_mul(
                                exm[:], ex[:], mask_sb[:, 384 - 128 * r: 896 - 128 * r])
                            ex = exm
                        nc.tensor.matmul(ps_at[:], v_sb[:, t, kv * D:(kv + 1) * D],
                                         ex[:], start=(t == 0), stop=(t == nblk - 1))
                        nc.tensor.matmul(ps_sm[:], ones_bf[:], ex[:],
                                         start=(t == 0), stop=(t == nblk - 1))
                    # delta: per-head shift correction
                    ps_dl = p_dl.tile([128, SQ], F32, tag="ps_dl")
                    nc.tensor.matmul(ps_dl[:], wsq_sb[:, h, :],
                                     qh_r, start=True, stop=False)
                    nc.tensor.matmul(ps_dl[:], wsk_sb[:, h, :],
                                     kT_sb[:, kv, sl],
                                     start=False, stop=True)
                    # normalize: attnT/sums + delta
                    rc = nrm_pool.tile([1, SQ], mybir.dt.float16, tag="rc")
                    with nc.allow_low_precision(reason="fp16 recip for bcast"):
                        nc.vector.reciprocal(rc[:], ps_sm[:])
                    ps_b = p_bc.tile([128, SQ], F32, tag="ps_b")
                    nc.tensor.matmul(ps_b[:], ones_f[:], rc[:],
                                     start=True, stop=True)
                    bc_sb = tmpB.tile([128, SQ], F32, tag="bc_sb")
                    nc.scalar.copy(bc_sb[:], ps_b[:])
                    t1 = tmpB.tile([128, SQ], F32, tag="t1")
                    nc.vector.tensor_mul(t1[:], ps_at[:], bc_sb[:])
                    nc.vector.tensor_add(attnT_sb[:, h, sl], t1[:], ps_dl[:])

        # ---------------- phase C: output projection ----------------
        with ExitStack() as cctx:
            wo_pool = cctx.enter_context(tc.tile_pool(name="wop", bufs=2))
            o_pool = cctx.enter_context(tc.tile_pool(name="op", bufs=4))
            p_o = cctx.enter_context(tc.tile_pool(name="p_o", bufs=4, space="PSUM"))
            for j in range(DM // 512):
                wo_sb = wo_pool.tile([128, HQ, 512], BF16, tag="wo")
                nc.sync.dma_start(wo_sb[:], wo_t[j])
                for m in range(S // 128):
                    ps = p_o.tile([128, 512], F32, tag="ps_o")
                    for t2 in range(HQ):
                        nc.tensor.matmul(ps[:], attnT_sb[:, t2, m * 128:(m + 1) * 128],
                                         wo_sb[:, t2, :],
                                         start=(t2 == 0), stop=(t2 == HQ - 1))
                    ot = o_pool.tile([128, 512], F32, tag="ot")
                    nc.scalar.copy(ot[:], ps[:])
                    nc.sync.dma_start(out[m * 128:(m + 1) * 128,
                                          j * 512:(j + 1) * 512], ot[:])
    _split_excess_waits(nc)
    return nc


# ---------------------------------------------------------------- host side
_CACHE = {}


def _prep_core_inputs(inputs, core):
    b, g = core // TP, core % TP
    hs = np.asarray(inputs["hidden_states"])[b]          # [S, DM] f32
    pos = np.asarray(inputs["position_ids"])[b]          # [S] int32
    Wq, Wk, Wv, Wo = (np.asarray(inputs[k]) for k in ("Wq", "Wk", "Wv", "Wo"))
    Ws_q, Ws_k = np.asarray(inputs["Ws_q"]), np.asarray(inputs["Ws_k"])

    qh0 = g * HQ                 # first global q head
    kv0 = g * HKV                # first global kv head

    inv_freq = 1.0 / (THETA ** (np.arange(0, D, 2, dtype=np.float64) / D))
    freqs = pos.astype(np.float64)[:, None] * inv_freq[None, :]   # [S, 64]
    cos = np.cos(freqs).astype(np.float32)
    sin = np.sin(freqs).astype(np.float32)
    cosT = np.ascontiguousarray(np.concatenate([cos, cos], axis=1).T)   # [128,S]
    sinsg = np.ascontiguousarray(np.concatenate([-sin, sin], axis=1).T)

    ii = np.arange(128)[:, None]
    cc = np.arange(896)[None, :]
    maskbig = ((cc - 384) >= ii).astype(BF_NP)

    # pre-tile into exact on-chip layouts (contiguous per-partition DMAs)
    hsT = hs.T.astype(BF_NP)                                   # [DM, S]
    hsT_t = np.ascontiguousarray(
        hsT.reshape(NK, 128, NSL, SQ).transpose(2, 1, 0, 3))   # [n, p, k, s]
    wq_c = Wq[:, qh0 * D:(qh0 + HQ) * D].astype(BF_NP)         # [DM, 1024]
    wq_t = np.ascontiguousarray(
        wq_c.reshape(NK, 128, HQ, D).transpose(2, 1, 0, 3))    # [h, p, k, m]
    wk_c = Wk[:, kv0 * D:(kv0 + HKV) * D].astype(BF_NP)
    wk_t = np.ascontiguousarray(
        wk_c.reshape(NK, 128, HKV * D).transpose(1, 0, 2))     # [p, k, m]
    wv_c = Wv[:, kv0 * D:(kv0 + HKV) * D].astype(BF_NP)
    wv_t = np.ascontiguousarray(
        wv_c.reshape(NK, 128, HKV * D).transpose(1, 0, 2))
    wo_c = Wo[qh0 * D:(qh0 + HQ) * D, :].astype(BF_NP)         # [1024, DM]
    wo_t = np.ascontiguousarray(
        wo_c.reshape(HQ, 128, DM // 512, 512).transpose(2, 1, 0, 3))  # [j,p,t,m]
    wsq_t = np.ascontiguousarray(
        Ws_q[qh0:qh0 + HQ].transpose(1, 0, 2)).astype(np.float32)  # [d, h, e]
    wsk_t = np.ascontiguousarray(
        Ws_k[qh0:qh0 + HQ].transpose(1, 0, 2)).astype(np.float32)
    return {
        "hsT_t": hsT_t,
        "wq_t": wq_t,
        "wk_t": wk_t,
        "wv_t": wv_t,
        "wo_t": wo_t,
        "wsq": wsq_t.astype(BF_NP),
        "wsk": wsk_t.astype(BF_NP),
        "cosT": cosT,
        "sinsg": sinsg,
        "maskbig": maskbig,
        "onesf": np.ones((1, 128), dtype=np.float16),
    }


def run(inputs, trace=False):
    if "nc" not in _CACHE:
        _CACHE["nc"] = build_kernel()
    nc = _CACHE["nc"]
    in_maps = [_prep_core_inputs(inputs, c) for c in range(N_CORES)]
    res = bass_utils.run_bass_kernel_spmd(
        nc, in_maps, core_ids=list(range(N_CORES)), trace=trace)
    full = np.zeros((B, S, DM), dtype=np.float32)
    for c in range(N_CORES):
        full[c // TP] += res.results[c]["out"]
    return full, res


def kernel(**inputs) -> np.ndarray:
    full, _ = run(inputs, trace=False)
    return full


# revision 43
# speedup vs baseline: 1.1170x; 1.0337x over previous
"""Trainium2 Bass kernel for nn_AttnApproximator (GQA attention + RoPE +
per-head shift correction), sharded over 8 NeuronCores.

Sharding: tensor-parallel over heads (4 groups of 8 query heads / 2 KV
heads) x data-parallel over batch (B=2) -> 8 cores. Each core computes a
partial output contribution [S, Dm] (its heads' slice of the attn @ Wo
contraction); the host sums the 4 head-group partials per batch element.

Per-core pipeline (everything stays transposed so no on-chip transposes
are needed):
  phase A: kT = (hs @ Wk).T with RoPE, v = hs @ Wv          (per s-slice)
  phase B: per s-slice of 512: qT = (hs @ Wq).T with RoPE, then causal
           attention per head in scores-transposed form:
             scoresT[sk, sq] = kT_tile.T @ qT    (f32r matmuls)
             expT = exp(scale * scoresT)          (ACT, bf16 out)
             attnT[dv, sq] += v_tile.T @ expT     (bf16 matmuls, PSUM acc)
             sums[1, sq]  += ones.T @ expT
           then delta = Ws_q.T-style per-head correction, normalize by
           broadcast(1/sums) via a K=1 PE matmul, add delta -> attnT(bf16)
  phase C: out[s, dm] = attnT.T @ Wo              (bf16 matmuls)
"""

import math
import numpy as np
import ml_dtypes

import bass_rust
import concourse.bass as bass
import concourse.tile as tile
from concourse import mybir
from concourse import bass_utils
from concourse.vector_clock import ScopedClock
from contextlib import ExitStack

# ---------------------------------------------------------------- constants
B, S, DM = 2, 2048, 4096
H, KV, D = 32, 8, 128
N_CORES = 8
TP = 4                    # head groups
HQ = H // TP              # 8 q heads per core
HKV = KV // TP            # 2 kv heads per core
GROUPS = H // KV          # 4
THETA = 10000.0
SQ = 512                  # s-slice width
NSL = S // SQ             # 4 slices
NK = DM // 128            # 32 contraction tiles
SCALE = 1.0 / math.sqrt(D)

F32 = mybir.dt.float32
F32R = mybir.dt.float32r
BF16 = mybir.dt.bfloat16
BF_NP = ml_dtypes.bfloat16


# ------------------------------------------------- walrus drain-wait fixup
class SplitDrainTileContext(tile.TileContext):
    """This container's walrus rejects >1 sync wait on the SP tail-drain
    CTRL instruction; split the gathered waits onto chained SP nops."""

    MAX_WAITS = 1

    def _drain_and_barrier(self, tick_clock, wait_clock):
        nc = self.nc
        drain_inst = nc.sync.drain()
        wait_clock.add_sem_waits(
            drain_inst.ins, ScopedClock({None: tick_clock.global_clock})
        )
        si = drain_inst.ins.sync_info
        waits = list(si.on_wait) if si is not None else []
        mw = self.MAX_WAITS
        if len(waits) > mw:
            drain_inst.ins.sync_info = bass_rust.SyncInfo(
                on_wait=waits[:mw], on_update=list(si.on_update)
            )
            for k in range(mw, len(waits), mw):
                nop = nc.sync.nop(nofuse=True, hint="drain_wait_split")
                nop.ins.sync_info = bass_rust.SyncInfo(
                    on_wait=waits[k : k + mw], on_update=[]
                )
        nc.all_engine_barrier()
        assert self.sems is not None
        popped = nc._tile_sem_poison_stack.pop()
        assert popped is self._sem_poison
        nc.clear_and_free_semaphores(list(self.sems.allocated().values()))
        nc.all_engine_barrier()


def _split_excess_waits(nc):
    """This walrus accepts 1 sync wait per instruction (2 for
    EventSemaphore). Tile emits more; move the excess onto same-engine
    NoOp carriers inserted immediately before the over-limit instruction."""
    uid = 0
    for fn in nc.m.functions:
        for bb in fn.blocks:
            new, changed = [], False
            for inst in bb.instructions:
                si = inst.sync_info
                waits = list(si.on_wait) if si is not None else []
                cap = 2 if inst.opcode == "EventSemaphore" else 1
                if len(waits) > cap:
                    changed = True
                    for w in waits[:-cap]:
                        nop = mybir.InstNoOp(
                            name=f"I-wsplit-{uid}",
                            engine=inst.engine,
                            bass_nofuse=True,
                            sync_info=mybir.SyncInfo(on_wait=[w], on_update=[]),
                        )
                        uid += 1
                        new.append(nop)
                    inst.sync_info = bass_rust.SyncInfo(
                        on_wait=waits[-cap:], on_update=list(si.on_update))
                new.append(inst)
            if changed:
                bb.instructions = new


# ---------------------------------------------------------------- builder
def _rope(nc, tmp_pool, out_ap, in_ps, cos_sl, sin_sl):
    """out = in*cosT + swap_halves(in)*sinT_signed ; in_ps is PSUM f32.
    First op drains PSUM via ACT so the bank frees fast (p_q bufs=1)."""
    q_sb = tmp_pool.tile([128, SQ], F32, tag="rope_q")
    nc.scalar.copy(q_sb[:], in_ps[:])
    sw = tmp_pool.tile([128, SQ], F32, tag="rope_sw")
    nc.vector.tensor_copy(sw[0:64, :], q_sb[64:128, :])
    nc.vector.tensor_copy(sw[64:128, :], q_sb[0:64, :])
    nc.vector.tensor_mul(sw[:], sw[:], sin_sl)
    t2 = tmp_pool.tile([128, SQ], F32, tag="rope_t2")
    nc.vector.tensor_mul(t2[:], q_sb[:], cos_sl)
    nc.vector.tensor_add(out_ap, t2[:], sw[:])


def build_kernel():
    nc = bass.Bass("TRN2", target_bir_lowering=False, debug=False,
                   num_devices=N_CORES)

    # All inputs are pre-tiled on the host into the exact sbuf layouts so
    # every DMA is contiguous per partition (few large descriptors).
    din = lambda n, shp, dt: nc.dram_tensor(n, shp, dt, kind="ExternalInput").ap()
    hsT_t = din("hsT_t", [NSL, 128, NK, SQ], BF16)
    wq_t = din("wq_t", [HQ, 128, NK, D], BF16)
    wk_t = din("wk_t", [128, NK, HKV * D], BF16)
    wv_t = din("wv_t", [128, NK, HKV * D], BF16)
    wo_t = din("wo_t", [DM // 512, 128, HQ, 512], BF16)
    wsq = din("wsq", [128, HQ, D], BF16)
    wsk = din("wsk", [128, HQ, D], BF16)
    cosT = din("cosT", [D, S], F32)
    sinsg = din("sinsg", [D, S], F32)
    maskbig = din("maskbig", [D, 896], BF16)
    onesf_in = din("onesf", [1, 128], mybir.dt.float16)
    out = nc.dram_tensor("out", [S, DM], F32, kind="ExternalOutput").ap()

    with SplitDrainTileContext(nc) as tc, ExitStack() as octx:
        # ---------------- persistent sbuf ----------------
        pers = octx.enter_context(tc.tile_pool(name="pers", bufs=1))
        kT_sb = pers.tile([128, HKV, S], BF16, tag="kT")         # 8KB/p
        v_sb = pers.tile([128, S // 128, HKV * D], BF16, tag="v")  # 8KB/p
        attnT_sb = pers.tile([128, HQ, S], BF16, tag="attnT")    # 32KB/p
        cos_sb = pers.tile([128, S], F32, tag="cos")             # 8KB/p
        sin_sb = pers.tile([128, S], F32, tag="sin")             # 8KB/p
        mask_sb = pers.tile([128, 896], BF16, tag="mask")
        wsq_sb = pers.tile([128, HQ, D], BF16, tag="wsq")        # 2KB/p
        wsk_sb = pers.tile([128, HQ, D], BF16, tag="wsk")
        ones_bf = pers.tile([128, 1], BF16, tag="ones_bf")
        ones_f = pers.tile([1, 128], mybir.dt.float16, tag="ones_f")

        nc.any.memset(ones_bf[:], 1.0)
        nc.sync.dma_start(ones_f[:], onesf_in[:, :])

        # ---------------- phase A: k/v projections ----------------
        with ExitStack() as actx:
            wkv_pool = actx.enter_context(tc.tile_pool(name="wkv", bufs=1))
            hstA_pool = actx.enter_context(tc.tile_pool(name="hstA", bufs=2))
            ptmpA = actx.enter_context(tc.tile_pool(name="ptmpA", bufs=3))
            psA = actx.enter_context(tc.tile_pool(name="psA", bufs=2, space="PSUM"))

            wk_sb = wkv_pool.tile([128, NK, HKV * D], BF16, tag="wk")  # 16KB/p
            wv_sb = wkv_pool.tile([128, NK, HKV * D], BF16, tag="wv")
            nc.sync.dma_start(wk_sb[:], wk_t[:, :, :])
            nc.sync.dma_start(wv_sb[:], wv_t[:, :, :])

            for n in range(NSL):
                sl = slice(n * SQ, (n + 1) * SQ)
                hst = hstA_pool.tile([128, NK, SQ], BF16, tag="hstA")
                nc.sync.dma_start(hst[:], hsT_t[n])
                if n == 0:
                    # issue after the critical-path loads so they don't
                    # steal startup DMA bandwidth
                    nc.sync.dma_start(cos_sb[:], cosT[:, :])
                    nc.sync.dma_start(sin_sb[:], sinsg[:, :])
                    nc.sync.dma_start(mask_sb[:], maskbig[:, :])
                    nc.sync.dma_start(wsq_sb[:], wsq[:, :, :])
                    nc.sync.dma_start(wsk_sb[:], wsk[:, :, :])
                # kT (transposed layout) + rope
                for kv in range(HKV):
                    ps = psA.tile([128, SQ], F32, tag="ps_k")
                    for k in range(NK):
                        nc.tensor.matmul(
                            ps[:], wk_sb[:, k, kv * D:(kv + 1) * D], hst[:, k, :],
                            start=(k == 0), stop=(k == NK - 1))
                    _rope(nc, ptmpA, kT_sb[:, kv, sl], ps,
                          cos_sb[:, sl], sin_sb[:, sl])
                # v natural layout [s, dv]
                for s4 in range(SQ // 128):
                    ps = psA.tile([128, HKV * D], F32, tag="ps_v")
                    for k in range(NK):
                        nc.tensor.matmul(
                            ps[:], hst[:, k, s4 * 128:(s4 + 1) * 128], wv_sb[:, k, :],
                            start=(k == 0), stop=(k == NK - 1))
                    nc.scalar.copy(v_sb[:, n * 4 + s4, :], ps[:])

        # ---------------- phase B: q proj + attention ----------------
        with ExitStack() as bctx:
            hstB_pool = bctx.enter_context(tc.tile_pool(name="hstB", bufs=1))
            wq_pool = bctx.enter_context(tc.tile_pool(name="wqp", bufs=2))
            qT_pool = bctx.enter_context(tc.tile_pool(name="qTp", bufs=2))
            exp_pool = bctx.enter_context(tc.tile_pool(name="expp", bufs=16))
            tmpB = bctx.enter_context(tc.tile_pool(name="tmpB", bufs=3))
            nrm_pool = bctx.enter_context(tc.tile_pool(name="nrmp", bufs=2))
            p_q = bctx.enter_context(tc.tile_pool(name="p_q", bufs=1, space="PSUM"))
            p_s = bctx.enter_context(tc.tile_pool(name="p_s", bufs=2, space="PSUM"))
            p_at = bctx.enter_context(tc.tile_pool(name="p_at", bufs=2, space="PSUM"))
            p_sm = bctx.enter_context(tc.tile_pool(name="p_sm", bufs=1, space="PSUM"))
            p_dl = bctx.enter_context(tc.tile_pool(name="p_dl", bufs=1, space="PSUM"))
            p_bc = bctx.enter_context(tc.tile_pool(name="p_bc", bufs=1, space="PSUM"))

            def _finish_head(st):
                """Deferred per-head epilogue: delta correction + normalize."""
                h, kv, hsl, hqT, ps_at, rc = st
                ps_dl = p_dl.tile([128, SQ], F32, tag="ps_dl")
                nc.tensor.matmul(ps_dl[:], wsq_sb[:, h, :], hqT[:, h, :],
                                 start=True, stop=False)
                nc.tensor.matmul(ps_dl[:], wsk_sb[:, h, :], kT_sb[:, kv, hsl],
                                 start=False, stop=True)
                ps_b = p_bc.tile([128, SQ], F32, tag="ps_b")
                nc.tensor.matmul(ps_b[:], ones_f[:], rc[:], start=True, stop=True)
                bc_sb = tmpB.tile([128, SQ], F32, tag="bc_sb")
                nc.scalar.copy(bc_sb[:], ps_b[:])
                t1 = tmpB.tile([128, SQ], F32, tag="t1")
                nc.vector.tensor_mul(t1[:], ps_at[:], bc_sb[:])
                nc.vector.tensor_add(attnT_sb[:, h, hsl], t1[:], ps_dl[:])

            prev = None
            for n in range(NSL):
                sl = slice(n * SQ, (n + 1) * SQ)
                hst = hstB_pool.tile([128, NK, SQ], BF16, tag="hstB")
                nc.sync.dma_start(hst[:], hsT_t[n])
                qT = qT_pool.tile([128, HQ, SQ], BF16, tag="qT")
                for h in range(HQ):
                    wqc = wq_pool.tile([128, NK, D], BF16, tag="wqc")
                    nc.sync.dma_start(wqc[:], wq_t[h])
                    ps = p_q.tile([128, SQ], F32, tag="ps_q")
                    for k in range(NK):
                        nc.tensor.matmul(ps[:], wqc[:, k, :], hst[:, k, :],
                                         start=(k == 0), stop=(k == NK - 1))
                    _rope(nc, tmpB, qT[:, h, :], ps, cos_sb[:, sl], sin_sb[:, sl])

                nblk = 4 * (n + 1)
                for h in range(HQ):
                    kv = h // GROUPS
                    qh_r = qT[:, h, :]
                    ps_at = p_at.tile([128, SQ], F32, tag="ps_at")
                    ps_sm = p_sm.tile([1, SQ], F32, tag="ps_sm")
                    exs = []
                    for t in range(nblk):
                        ps_sc = p_s.tile([128, SQ], F32, tag="ps_sc")
                        nc.tensor.matmul(
                            ps_sc[:],
                            kT_sb[:, kv, t * 128:(t + 1) * 128],
                            qh_r, start=True, stop=True)
                        ex = exp_pool.tile([128, SQ], BF16, tag="ex")
                        nc.scalar.activation(ex[:], ps_sc[:],
                                             mybir.ActivationFunctionType.Exp,
                                             scale=SCALE)
                        if t >= 4 * n:
                            r = t - 4 * n
                            exm = exp_pool.tile([128, SQ], BF16, tag="exm")
                            nc.vector.tensor_mul(
                                exm[:], ex[:], mask_sb[:, 384 - 128 * r: 896 - 128 * r])
                            ex = exm
                        nc.tensor.matmul(ps_at[:], v_sb[:, t, kv * D:(kv + 1) * D],
                                         ex[:], start=(t == 0), stop=(t == nblk - 1))
                        exs.append(ex)
                    # sums as an end-burst so ps_sm frees early next head
                    for t, ex in enumerate(exs):
                        nc.tensor.matmul(ps_sm[:], ones_bf[:], ex[:],
                                         start=(t == 0), stop=(t == nblk - 1))
                    rc = nrm_pool.tile([1, SQ], mybir.dt.float16, tag="rc")
                    with nc.allow_low_precision(reason="fp16 recip for bcast"):
                        nc.vector.reciprocal(rc[:], ps_sm[:])
                    # normalize the PREVIOUS head now: its reciprocal has had a
                    # full head of PE work to finish, so PE never waits on DVE
                    if prev is not None:
                        _finish_head(prev)
                    prev = (h, kv, sl, qT, ps_at, rc)
            _finish_head(prev)

        # ---------------- phase C: output projection ----------------
        with ExitStack() as cctx:
            wo_pool = cctx.enter_context(tc.tile_pool(name="wop", bufs=2))
            o_pool = cctx.enter_context(tc.tile_pool(name="op", bufs=4))
            p_o = cctx.enter_context(tc.tile_pool(name="p_o", bufs=4, space="PSUM"))
            for j in range(DM // 512):
                wo_sb = wo_pool.tile([128, HQ, 512], BF16, tag="wo")
                nc.sync.dma_start(wo_sb[:], wo_t[j])
                for m in range(S // 128):
                    ps = p_o.tile([128, 512], F32, tag="ps_o")
                    for t2 in range(HQ):
                        nc.tensor.matmul(ps[:], attnT_sb[:, t2, m * 128:(m + 1) * 128],
                                         wo_sb[:, t2, :],
                                         start=(t2 == 0), stop=(t2 == HQ - 1))
                    ot = o_pool.tile([128, 512], F32, tag="ot")
                    nc.scalar.copy(ot[:], ps[:])
                    nc.sync.dma_start(out[m * 128:(m + 1) * 128,
                                          j * 512:(j + 1) * 512], ot[:])
    _split_excess_waits(nc)
    return nc


# ---------------------------------------------------------------- host side
_CACHE = {}


def _prep_core_inputs(inputs, core):
    b, g = core // TP, core % TP
    hs = np.asarray(inputs["hidden_states"])[b]          # [S, DM] f32
    pos = np.asarray(inputs["position_ids"])[b]          # [S] int32
    Wq, Wk, Wv, Wo = (np.asarray(inputs[k]) for k in ("Wq", "Wk", "Wv", "Wo"))
    Ws_q, Ws_k = np.asarray(inputs["Ws_q"]), np.asarray(inputs["Ws_k"])

    qh0 = g * HQ                 # first global q head
    kv0 = g * HKV                # first global kv head

    inv_freq = 1.0 / (THETA ** (np.arange(0, D, 2, dtype=np.float64) / D))
    freqs = pos.astype(np.float64)[:, None] * inv_freq[None, :]   # [S, 64]
    cos = np.cos(freqs).astype(np.float32)
    sin = np.sin(freqs).astype(np.float32)
    cosT = np.ascontiguousarray(np.concatenate([cos, cos], axis=1).T)   # [128,S]
    sinsg = np.ascontiguousarray(np.concatenate([-sin, sin], axis=1).T)

    ii = np.arange(128)[:, None]
    cc = np.arange(896)[None, :]
    maskbig = ((cc - 384) >= ii).astype(BF_NP)

    # pre-tile into exact on-chip layouts (contiguous per-partition DMAs)
    hsT = hs.T.astype(BF_NP)                                   # [DM, S]
    hsT_t = np.ascontiguousarray(
        hsT.reshape(NK, 128, NSL, SQ).transpose(2, 1, 0, 3))   # [n, p, k, s]
    wq_c = Wq[:, qh0 * D:(qh0 + HQ) * D].astype(BF_NP)         # [DM, 1024]
    wq_t = np.ascontiguousarray(
        wq_c.reshape(NK, 128, HQ, D).transpose(2, 1, 0, 3))    # [h, p, k, m]
    wk_c = Wk[:, kv0 * D:(kv0 + HKV) * D].astype(BF_NP)
    wk_t = np.ascontiguousarray(
        wk_c.reshape(NK, 128, HKV * D).transpose(1, 0, 2))     # [p, k, m]
    wv_c = Wv[:, kv0 * D:(kv0 + HKV) * D].astype(BF_NP)
    wv_t = np.ascontiguousarray(
        wv_c.reshape(NK, 128, HKV * D).transpose(1, 0, 2))
    wo_c = Wo[qh0 * D:(qh0 + HQ) * D, :].astype(BF_NP)         # [1024, DM]
    wo_t = np.ascontiguousarray(
        wo_c.reshape(HQ, 128, DM // 512, 512).transpose(2, 1, 0, 3))  # [j,p,t,m]
    wsq_t = np.ascontiguousarray(
        Ws_q[qh0:qh0 + HQ].transpose(1, 0, 2)).astype(np.float32)  # [d, h, e]
    wsk_t = np.ascontiguousarray(
        Ws_k[qh0:qh0 + HQ].transpose(1, 0, 2)).astype(np.float32)
    return {
        "hsT_t": hsT_t,
        "wq_t": wq_t,
        "wk_t": wk_t,
        "wv_t": wv_t,
        "wo_t": wo_t,
        "wsq": wsq_t.astype(BF_NP),
        "wsk": wsk_t.astype(BF_NP),
        "cosT": cosT,
        "sinsg": sinsg,
        "maskbig": maskbig,
        "onesf": np.ones((1, 128), dtype=np.float16),
    }


def run(inputs, trace=False):
    if "nc" not in _CACHE:
        _CACHE["nc"] = build_kernel()
    nc = _CACHE["nc"]
    in_maps = [_prep_core_inputs(inputs, c) for c in range(N_CORES)]
    res = bass_utils.run_bass_kernel_spmd(
        nc, in_maps, core_ids=list(range(N_CORES)), trace=trace)
    full = np.zeros((B, S, DM), dtype=np.float32)
    for c in range(N_CORES):
        full[c // TP] += res.results[c]["out"]
    return full, res


def kernel(**inputs) -> np.ndarray:
    full, _ = run(inputs, trace=False)
    return full


# revision 45
# speedup vs baseline: 1.1241x; 1.0063x over previous
"""Trainium2 Bass kernel for nn_AttnApproximator (GQA attention + RoPE +
per-head shift correction), sharded over 8 NeuronCores.

Sharding: tensor-parallel over heads (4 groups of 8 query heads / 2 KV
heads) x data-parallel over batch (B=2) -> 8 cores. Each core computes a
partial output contribution [S, Dm] (its heads' slice of the attn @ Wo
contraction); the host sums the 4 head-group partials per batch element.

Per-core pipeline (everything stays transposed so no on-chip transposes
are needed):
  phase A: kT = (hs @ Wk).T with RoPE, v = hs @ Wv          (per s-slice)
  phase B: per s-slice of 512: qT = (hs @ Wq).T with RoPE, then causal
           attention per head in scores-transposed form:
             scoresT[sk, sq] = kT_tile.T @ qT    (f32r matmuls)
             expT = exp(scale * scoresT)          (ACT, bf16 out)
             attnT[dv, sq] += v_tile.T @ expT     (bf16 matmuls, PSUM acc)
             sums[1, sq]  += ones.T @ expT
           then delta = Ws_q.T-style per-head correction, normalize by
           broadcast(1/sums) via a K=1 PE matmul, add delta -> attnT(bf16)
  phase C: out[s, dm] = attnT.T @ Wo              (bf16 matmuls)
"""

import math
import numpy as np
import ml_dtypes

import bass_rust
import concourse.bass as bass
import concourse.tile as tile
from concourse import mybir
from concourse import bass_utils
from concourse.vector_clock import ScopedClock
from contextlib import ExitStack

# ---------------------------------------------------------------- constants
B, S, DM = 2, 2048, 4096
H, KV, D = 32, 8, 128
N_CORES = 8
TP = 4                    # head groups
HQ = H // TP              # 8 q heads per core
HKV = KV // TP            # 2 kv heads per core
GROUPS = H // KV          # 4
THETA = 10000.0
SQ = 512                  # s-slice width
NSL = S // SQ             # 4 slices
NK = DM // 128            # 32 contraction tiles
SCALE = 1.0 / math.sqrt(D)

F32 = mybir.dt.float32
F32R = mybir.dt.float32r
BF16 = mybir.dt.bfloat16
BF_NP = ml_dtypes.bfloat16


# ------------------------------------------------- walrus drain-wait fixup
class SplitDrainTileContext(tile.TileContext):
    """This container's walrus rejects >1 sync wait on the SP tail-drain
    CTRL instruction; split the gathered waits onto chained SP nops."""

    MAX_WAITS = 1

    def _drain_and_barrier(self, tick_clock, wait_clock):
        nc = self.nc
        drain_inst = nc.sync.drain()
        wait_clock.add_sem_waits(
            drain_inst.ins, ScopedClock({None: tick_clock.global_clock})
        )
        si = drain_inst.ins.sync_info
        waits = list(si.on_wait) if si is not None else []
        mw = self.MAX_WAITS
        if len(waits) > mw:
            drain_inst.ins.sync_info = bass_rust.SyncInfo(
                on_wait=waits[:mw], on_update=list(si.on_update)
            )
            for k in range(mw, len(waits), mw):
                nop = nc.sync.nop(nofuse=True, hint="drain_wait_split")
                nop.ins.sync_info = bass_rust.SyncInfo(
                    on_wait=waits[k : k + mw], on_update=[]
                )
        nc.all_engine_barrier()
        assert self.sems is not None
        popped = nc._tile_sem_poison_stack.pop()
        assert popped is self._sem_poison
        nc.clear_and_free_semaphores(list(self.sems.allocated().values()))
        nc.all_engine_barrier()


def _split_excess_waits(nc):
    """This walrus accepts 1 sync wait per instruction (2 for
    EventSemaphore). Tile emits more; move the excess onto same-engine
    NoOp carriers inserted immediately before the over-limit instruction."""
    uid = 0
    for fn in nc.m.functions:
        for bb in fn.blocks:
            new, changed = [], False
            for inst in bb.instructions:
                si = inst.sync_info
                waits = list(si.on_wait) if si is not None else []
                cap = 2 if inst.opcode == "EventSemaphore" else 1
                if len(waits) > cap:
                    changed = True
                    for w in waits[:-cap]:
                        nop = mybir.InstNoOp(
                            name=f"I-wsplit-{uid}",
                            engine=inst.engine,
                            bass_nofuse=True,
                            sync_info=mybir.SyncInfo(on_wait=[w], on_update=[]),
                        )
                        uid += 1
                        new.append(nop)
                    inst.sync_info = bass_rust.SyncInfo(
                        on_wait=waits[-cap:], on_update=list(si.on_update))
                new.append(inst)
            if changed:
                bb.instructions = new


# ---------------------------------------------------------------- builder
def _rope(nc, tmp_pool, out_ap, in_ps, cos_sl, sin_sl):
    """out = in*cosT + swap_halves(in)*sinT_signed ; in_ps is PSUM f32.
    First op drains PSUM via ACT so the bank frees fast (p_q bufs=1)."""
    q_sb = tmp_pool.tile([128, SQ], F32, tag="rope_q")
    nc.scalar.copy(q_sb[:], in_ps[:])
    sw = tmp_pool.tile([128, SQ], F32, tag="rope_sw")
    nc.vector.tensor_copy(sw[0:64, :], q_sb[64:128, :])
    nc.vector.tensor_copy(sw[64:128, :], q_sb[0:64, :])
    nc.vector.tensor_mul(sw[:], sw[:], sin_sl)
    t2 = tmp_pool.tile([128, SQ], F32, tag="rope_t2")
    nc.vector.tensor_mul(t2[:], q_sb[:], cos_sl)
    nc.vector.tensor_add(out_ap, t2[:], sw[:])


def build_kernel():
    nc = bass.Bass("TRN2", target_bir_lowering=False, debug=False,
                   num_devices=N_CORES)

    # All inputs are pre-tiled on the host into the exact sbuf layouts so
    # every DMA is contiguous per partition (few large descriptors).
    din = lambda n, shp, dt: nc.dram_tensor(n, shp, dt, kind="ExternalInput").ap()
    hsT_t = din("hsT_t", [NSL, 128, NK, SQ], BF16)
    wq_t = din("wq_t", [HQ, 128, NK, D], BF16)
    wk_t = din("wk_t", [128, NK, HKV * D], BF16)
    wv_t = din("wv_t", [128, NK, HKV * D], BF16)
    wo_t = din("wo_t", [DM // 512, 128, HQ, 512], BF16)
    wsq = din("wsq", [128, HQ, D], BF16)
    wsk = din("wsk", [128, HQ, D], BF16)
    cosT = din("cosT", [D, S], F32)
    sinsg = din("sinsg", [D, S], F32)
    maskbig = din("maskbig", [D, 896], BF16)
    onesf_in = din("onesf", [1, 128], mybir.dt.float16)
    out = nc.dram_tensor("out", [S, DM], F32, kind="ExternalOutput").ap()

    with SplitDrainTileContext(nc) as tc, ExitStack() as octx:
        # ---------------- persistent sbuf ----------------
        pers = octx.enter_context(tc.tile_pool(name="pers", bufs=1))
        kT_sb = pers.tile([128, HKV, S], BF16, tag="kT")         # 8KB/p
        v_sb = pers.tile([128, S // 128, HKV * D], BF16, tag="v")  # 8KB/p
        attnT_sb = pers.tile([128, HQ, S], BF16, tag="attnT")    # 32KB/p
        cos_sb = pers.tile([128, S], F32, tag="cos")             # 8KB/p
        sin_sb = pers.tile([128, S], F32, tag="sin")             # 8KB/p
        mask_sb = pers.tile([128, 896], BF16, tag="mask")
        wsq_sb = pers.tile([128, HQ, D], BF16, tag="wsq")        # 2KB/p
        wsk_sb = pers.tile([128, HQ, D], BF16, tag="wsk")
        ones_bf = pers.tile([128, 1], BF16, tag="ones_bf")
        ones_f = pers.tile([1, 128], mybir.dt.float16, tag="ones_f")

        nc.any.memset(ones_bf[:], 1.0)
        nc.sync.dma_start(ones_f[:], onesf_in[:, :])

        # ---------------- phase A: k/v projections ----------------
        with ExitStack() as actx:
            wkv_pool = actx.enter_context(tc.tile_pool(name="wkv", bufs=1))
            hstA_pool = actx.enter_context(tc.tile_pool(name="hstA", bufs=2))
            ptmpA = actx.enter_context(tc.tile_pool(name="ptmpA", bufs=3))
            psA = actx.enter_context(tc.tile_pool(name="psA", bufs=2, space="PSUM"))

            wk_sb = wkv_pool.tile([128, NK, HKV * D], BF16, tag="wk")  # 16KB/p
            wv_sb = wkv_pool.tile([128, NK, HKV * D], BF16, tag="wv")
            nc.sync.dma_start(wk_sb[:], wk_t[:, :, :])
            nc.sync.dma_start(wv_sb[:], wv_t[:, :, :])

            for n in range(NSL):
                sl = slice(n * SQ, (n + 1) * SQ)
                hst = hstA_pool.tile([128, NK, SQ], BF16, tag="hstA")
                nc.sync.dma_start(hst[:], hsT_t[n])
                if n == 0:
                    # issue after the critical-path loads so they don't
                    # steal startup DMA bandwidth
                    nc.sync.dma_start(cos_sb[:], cosT[:, :])
                    nc.sync.dma_start(sin_sb[:], sinsg[:, :])
                    nc.sync.dma_start(mask_sb[:], maskbig[:, :])
                    nc.sync.dma_start(wsq_sb[:], wsq[:, :, :])
                    nc.sync.dma_start(wsk_sb[:], wsk[:, :, :])
                # kT (transposed layout) + rope
                for kv in range(HKV):
                    ps = psA.tile([128, SQ], F32, tag="ps_k")
                    for k in range(NK):
                        nc.tensor.matmul(
                            ps[:], wk_sb[:, k, kv * D:(kv + 1) * D], hst[:, k, :],
                            start=(k == 0), stop=(k == NK - 1))
                    _rope(nc, ptmpA, kT_sb[:, kv, sl], ps,
                          cos_sb[:, sl], sin_sb[:, sl])
                # v natural layout [s, dv]
                for s4 in range(SQ // 128):
                    ps = psA.tile([128, HKV * D], F32, tag="ps_v")
                    for k in range(NK):
                        nc.tensor.matmul(
                            ps[:], hst[:, k, s4 * 128:(s4 + 1) * 128], wv_sb[:, k, :],
                            start=(k == 0), stop=(k == NK - 1))
                    nc.scalar.copy(v_sb[:, n * 4 + s4, :], ps[:])

        # ---------------- phase B: q proj + attention ----------------
        with ExitStack() as bctx:
            hstB_pool = bctx.enter_context(tc.tile_pool(name="hstB", bufs=1))
            wq_pool = bctx.enter_context(tc.tile_pool(name="wqp", bufs=2))
            qT_pool = bctx.enter_context(tc.tile_pool(name="qTp", bufs=2))
            exp_pool = bctx.enter_context(tc.tile_pool(name="expp", bufs=16))
            tmpB = bctx.enter_context(tc.tile_pool(name="tmpB", bufs=3))
            nrm_pool = bctx.enter_context(tc.tile_pool(name="nrmp", bufs=3))
            p_q = bctx.enter_context(tc.tile_pool(name="p_q", bufs=1, space="PSUM"))
            p_s = bctx.enter_context(tc.tile_pool(name="p_s", bufs=2, space="PSUM"))
            p_at = bctx.enter_context(tc.tile_pool(name="p_at", bufs=2, space="PSUM"))
            p_sm = bctx.enter_context(tc.tile_pool(name="p_sm", bufs=1, space="PSUM"))
            p_dl = bctx.enter_context(tc.tile_pool(name="p_dl", bufs=1, space="PSUM"))
            p_bc = bctx.enter_context(tc.tile_pool(name="p_bc", bufs=1, space="PSUM"))

            def _finish_head(st):
                """Deferred per-head epilogue: delta correction + normalize."""
                h, kv, hsl, hqT, ps_at, rc = st
                ps_dl = p_dl.tile([128, SQ], F32, tag="ps_dl")
                nc.tensor.matmul(ps_dl[:], wsq_sb[:, h, :], hqT[:, h, :],
                                 start=True, stop=False)
                nc.tensor.matmul(ps_dl[:], wsk_sb[:, h, :], kT_sb[:, kv, hsl],
                                 start=False, stop=True)
                ps_b = p_bc.tile([128, SQ], F32, tag="ps_b")
                nc.tensor.matmul(ps_b[:], ones_f[:], rc[:], start=True, stop=True)
                bc_sb = tmpB.tile([128, SQ], F32, tag="bc_sb")
                nc.vector.tensor_copy(bc_sb[:], ps_b[:])
                t1 = tmpB.tile([128, SQ], F32, tag="t1")
                nc.vector.tensor_mul(t1[:], ps_at[:], bc_sb[:])
                nc.vector.tensor_add(attnT_sb[:, h, hsl], t1[:], ps_dl[:])

            prev = None
            for n in range(NSL):
                sl = slice(n * SQ, (n + 1) * SQ)
                hst = hstB_pool.tile([128, NK, SQ], BF16, tag="hstB")
                nc.sync.dma_start(hst[:], hsT_t[n])
                qT = qT_pool.tile([128, HQ, SQ], BF16, tag="qT")
                for h in range(HQ):
                    wqc = wq_pool.tile([128, NK, D], BF16, tag="wqc")
                    nc.sync.dma_start(wqc[:], wq_t[h])
                    ps = p_q.tile([128, SQ], F32, tag="ps_q")
                    for k in range(NK):
                        nc.tensor.matmul(ps[:], wqc[:, k, :], hst[:, k, :],
                                         start=(k == 0), stop=(k == NK - 1))
                    _rope(nc, tmpB, qT[:, h, :], ps, cos_sb[:, sl], sin_sb[:, sl])

                nblk = 4 * (n + 1)
                for h in range(HQ):
                    kv = h // GROUPS
                    qh_r = qT[:, h, :]
                    ps_at = p_at.tile([128, SQ], F32, tag="ps_at")
                    ps_sm = p_sm.tile([1, SQ], F32, tag="ps_sm")
                    exs = []
                    for t in range(nblk):
                        ps_sc = p_s.tile([128, SQ], F32, tag="ps_sc")
                        nc.tensor.matmul(
                            ps_sc[:],
                            kT_sb[:, kv, t * 128:(t + 1) * 128],
                            qh_r, start=True, stop=True)
                        ex = exp_pool.tile([128, SQ], BF16, tag="ex")
                        nc.scalar.activation(ex[:], ps_sc[:],
                                             mybir.ActivationFunctionType.Exp,
                                             scale=SCALE)
                        if t >= 4 * n:
                            r = t - 4 * n
                            exm = exp_pool.tile([128, SQ], BF16, tag="exm")
                            nc.vector.tensor_mul(
                                exm[:], ex[:], mask_sb[:, 384 - 128 * r: 896 - 128 * r])
                            ex = exm
                        nc.tensor.matmul(ps_at[:], v_sb[:, t, kv * D:(kv + 1) * D],
                                         ex[:], start=(t == 0), stop=(t == nblk - 1))
                        exs.append(ex)
                    # sums as an end-burst so ps_sm frees early next head
                    for t, ex in enumerate(exs):
                        nc.tensor.matmul(ps_sm[:], ones_bf[:], ex[:],
                                         start=(t == 0), stop=(t == nblk - 1))
                    rc = nrm_pool.tile([1, SQ], mybir.dt.float16, tag="rc")
                    with nc.allow_low_precision(reason="fp16 recip for bcast"):
                        nc.vector.reciprocal(rc[:], ps_sm[:])
                    # normalize the PREVIOUS head now: its reciprocal has had a
                    # full head of PE work to finish, so PE never waits on DVE
                    if prev is not None:
                        _finish_head(prev)
                    prev = (h, kv, sl, qT, ps_at, rc)
            _finish_head(prev)

        # ---------------- phase C: output projection ----------------
        with ExitStack() as cctx:
            wo_pool = cctx.enter_context(tc.tile_pool(name="wop", bufs=2))
            o_pool = cctx.enter_context(tc.tile_pool(name="op", bufs=4))
            p_o = cctx.enter_context(tc.tile_pool(name="p_o", bufs=4, space="PSUM"))
            for j in range(DM // 512):
                wo_sb = wo_pool.tile([128, HQ, 512], BF16, tag="wo")
                nc.sync.dma_start(wo_sb[:], wo_t[j])
                for m in range(S // 128):
                    ps = p_o.tile([128, 512], F32, tag="ps_o")
                    for t2 in range(HQ):
                        nc.tensor.matmul(ps[:], attnT_sb[:, t2, m * 128:(m + 1) * 128],
                                         wo_sb[:, t2, :],
                                         start=(t2 == 0), stop=(t2 == HQ - 1))
                    ot = o_pool.tile([128, 512], F32, tag="ot")
                    nc.scalar.copy(ot[:], ps[:])
                    nc.sync.dma_start(out[m * 128:(m + 1) * 128,
                                          j * 512:(j + 1) * 512], ot[:])
    _split_excess_waits(nc)
    return nc


# ---------------------------------------------------------------- host side
_CACHE = {}


def _prep_core_inputs(inputs, core):
    b, g = core // TP, core % TP
    hs = np.asarray(inputs["hidden_states"])[b]          # [S, DM] f32
    pos = np.asarray(inputs["position_ids"])[b]          # [S] int32
    Wq, Wk, Wv, Wo = (np.asarray(inputs[k]) for k in ("Wq", "Wk", "Wv", "Wo"))
    Ws_q, Ws_k = np.asarray(inputs["Ws_q"]), np.asarray(inputs["Ws_k"])

    qh0 = g * HQ                 # first global q head
    kv0 = g * HKV                # first global kv head

    inv_freq = 1.0 / (THETA ** (np.arange(0, D, 2, dtype=np.float64) / D))
    freqs = pos.astype(np.float64)[:, None] * inv_freq[None, :]   # [S, 64]
    cos = np.cos(freqs).astype(np.float32)
    sin = np.sin(freqs).astype(np.float32)
    cosT = np.ascontiguousarray(np.concatenate([cos, cos], axis=1).T)   # [128,S]
    sinsg = np.ascontiguousarray(np.concatenate([-sin, sin], axis=1).T)

    ii = np.arange(128)[:, None]
    cc = np.arange(896)[None, :]
    maskbig = ((cc - 384) >= ii).astype(BF_NP)

    # pre-tile into exact on-chip layouts (contiguous per-partition DMAs)
    hsT = hs.T.astype(BF_NP)                                   # [DM, S]
    hsT_t = np.ascontiguousarray(
        hsT.reshape(NK, 128, NSL, SQ).transpose(2, 1, 0, 3))   # [n, p, k, s]
    wq_c = Wq[:, qh0 * D:(qh0 + HQ) * D].astype(BF_NP)         # [DM, 1024]
    wq_t = np.ascontiguousarray(
        wq_c.reshape(NK, 128, HQ, D).transpose(2, 1, 0, 3))    # [h, p, k, m]
    wk_c = Wk[:, kv0 * D:(kv0 + HKV) * D].astype(BF_NP)
    wk_t = np.ascontiguousarray(
        wk_c.reshape(NK, 128, HKV * D).transpose(1, 0, 2))     # [p, k, m]
    wv_c = Wv[:, kv0 * D:(kv0 + HKV) * D].astype(BF_NP)
    wv_t = np.ascontiguousarray(
        wv_c.reshape(NK, 128, HKV * D).transpose(1, 0, 2))
    wo_c = Wo[qh0 * D:(qh0 + HQ) * D, :].astype(BF_NP)         # [1024, DM]
    wo_t = np.ascontiguousarray(
        wo_c.reshape(HQ, 128, DM // 512, 512).transpose(2, 1, 0, 3))  # [j,p,t,m]
    wsq_t = np.ascontiguousarray(
        Ws_q[qh0:qh0 + HQ].transpose(1, 0, 2)).astype(np.float32)  # [d, h, e]
    wsk_t = np.ascontiguousarray(
        Ws_k[qh0:qh0 + HQ].transpose(1, 0, 2)).astype(np.float32)
    return {
        "hsT_t": hsT_t,
        "wq_t": wq_t,
        "wk_t": wk_t,
        "wv_t": wv_t,
        "wo_t": wo_t,
        "wsq": wsq_t.astype(BF_NP),
        "wsk": wsk_t.astype(BF_NP),
        "cosT": cosT,
        "sinsg": sinsg,
        "maskbig": maskbig,
        "onesf": np.ones((1, 128), dtype=np.float16),
    }


def run(inputs, trace=False):
    if "nc" not in _CACHE:
        _CACHE["nc"] = build_kernel()
    nc = _CACHE["nc"]
    in_maps = [_prep_core_inputs(inputs, c) for c in range(N_CORES)]
    res = bass_utils.run_bass_kernel_spmd(
        nc, in_maps, core_ids=list(range(N_CORES)), trace=trace)
    full = np.zeros((B, S, DM), dtype=np.float32)
    for c in range(N_CORES):
        full[c // TP] += res.results[c]["out"]
    return full, res


def kernel(**inputs) -> np.ndarray:
    full, _ = run(inputs, trace=False)
    return full


# revision 52
# speedup vs baseline: 1.2282x; 1.0926x over previous
"""Trainium2 Bass kernel for nn_AttnApproximator (GQA attention + RoPE +
per-head shift correction), sharded over 8 NeuronCores.

Sharding: tensor-parallel over heads (4 groups of 8 query heads / 2 KV
heads) x data-parallel over batch (B=2) -> 8 cores. Each core computes a
partial output contribution [S, Dm] (its heads' slice of the attn @ Wo
contraction); the host sums the 4 head-group partials per batch element.

Per-core pipeline (everything stays transposed so no on-chip transposes
are needed):
  phase A: kT = (hs @ Wk).T with RoPE, v = hs @ Wv          (per s-slice)
  phase B: per s-slice of 512: qT = (hs @ Wq).T with RoPE, then causal
           attention per head in scores-transposed form:
             scoresT[sk, sq] = kT_tile.T @ qT    (f32r matmuls)
             expT = exp(scale * scoresT)          (ACT, bf16 out)
             attnT[dv, sq] += v_tile.T @ expT     (bf16 matmuls, PSUM acc)
             sums[1, sq]  += ones.T @ expT
           then delta = Ws_q.T-style per-head correction, normalize by
           broadcast(1/sums) via a K=1 PE matmul, add delta -> attnT(bf16)
  phase C: out[s, dm] = attnT.T @ Wo              (bf16 matmuls)
"""

import math
import numpy as np
import ml_dtypes

import bass_rust
import concourse.bass as bass
import concourse.tile as tile
from concourse import mybir
from concourse import bass_utils
from concourse.vector_clock import ScopedClock
from contextlib import ExitStack

# ---------------------------------------------------------------- constants
B, S, DM = 2, 2048, 4096
H, KV, D = 32, 8, 128
N_CORES = 8
TP = 4                    # head groups
HQ = H // TP              # 8 q heads per core
HKV = KV // TP            # 2 kv heads per core
GROUPS = H // KV          # 4
THETA = 10000.0
SQ = 512                  # s-slice width
NSL = S // SQ             # 4 slices
NK = DM // 128            # 32 contraction tiles
SCALE = 1.0 / math.sqrt(D)

F32 = mybir.dt.float32
F32R = mybir.dt.float32r
BF16 = mybir.dt.bfloat16
BF_NP = ml_dtypes.bfloat16


# ------------------------------------------------- walrus drain-wait fixup
class SplitDrainTileContext(tile.TileContext):
    """This container's walrus rejects >1 sync wait on the SP tail-drain
    CTRL instruction; split the gathered waits onto chained SP nops."""

    MAX_WAITS = 1

    def _drain_and_barrier(self, tick_clock, wait_clock):
        nc = self.nc
        drain_inst = nc.sync.drain()
        wait_clock.add_sem_waits(
            drain_inst.ins, ScopedClock({None: tick_clock.global_clock})
        )
        si = drain_inst.ins.sync_info
        waits = list(si.on_wait) if si is not None else []
        mw = self.MAX_WAITS
        if len(waits) > mw:
            drain_inst.ins.sync_info = bass_rust.SyncInfo(
                on_wait=waits[:mw], on_update=list(si.on_update)
            )
            for k in range(mw, len(waits), mw):
                nop = nc.sync.nop(nofuse=True, hint="drain_wait_split")
                nop.ins.sync_info = bass_rust.SyncInfo(
                    on_wait=waits[k : k + mw], on_update=[]
                )
        nc.all_engine_barrier()
        assert self.sems is not None
        popped = nc._tile_sem_poison_stack.pop()
        assert popped is self._sem_poison
        nc.clear_and_free_semaphores(list(self.sems.allocated().values()))
        nc.all_engine_barrier()


def _split_excess_waits(nc):
    """This walrus accepts 1 sync wait per instruction (2 for
    EventSemaphore). Tile emits more; move the excess onto same-engine
    NoOp carriers inserted immediately before the over-limit instruction."""
    uid = 0
    for fn in nc.m.functions:
        for bb in fn.blocks:
            new, changed = [], False
            for inst in bb.instructions:
                si = inst.sync_info
                waits = list(si.on_wait) if si is not None else []
                cap = 2 if inst.opcode == "EventSemaphore" else 1
                if len(waits) > cap:
                    changed = True
                    for w in waits[:-cap]:
                        nop = mybir.InstNoOp(
                            name=f"I-wsplit-{uid}",
                            engine=inst.engine,
                            bass_nofuse=True,
                            sync_info=mybir.SyncInfo(on_wait=[w], on_update=[]),
                        )
                        uid += 1
                        new.append(nop)
                    inst.sync_info = bass_rust.SyncInfo(
                        on_wait=waits[-cap:], on_update=list(si.on_update))
                new.append(inst)
            if changed:
                bb.instructions = new


# ---------------------------------------------------------------- builder
def _rope(nc, tmp_pool, out_ap, in_ps, cos_sl, sin_sl):
    """out = in*cosT + swap_halves(in)*sinT_signed ; in_ps is PSUM f32.
    First op drains PSUM via ACT so the bank frees fast (p_q bufs=1)."""
    q_sb = tmp_pool.tile([128, SQ], F32, tag="rope_q")
    nc.scalar.copy(q_sb[:], in_ps[:])
    sw = tmp_pool.tile([128, SQ], F32, tag="rope_sw")
    nc.vector.tensor_copy(sw[0:64, :], q_sb[64:128, :])
    nc.vector.tensor_copy(sw[64:128, :], q_sb[0:64, :])
    nc.vector.tensor_mul(sw[:], sw[:], sin_sl)
    t2 = tmp_pool.tile([128, SQ], F32, tag="rope_t2")
    nc.vector.tensor_mul(t2[:], q_sb[:], cos_sl)
    nc.vector.tensor_add(out_ap, t2[:], sw[:])


def build_kernel():
    nc = bass.Bass("TRN2", target_bir_lowering=False, debug=False,
                   num_devices=N_CORES)

    # All inputs are pre-tiled on the host into the exact sbuf layouts so
    # every DMA is contiguous per partition (few large descriptors).
    din = lambda n, shp, dt: nc.dram_tensor(n, shp, dt, kind="ExternalInput").ap()
    hsT_t = din("hsT_t", [NSL, 128, NK, SQ], BF16)
    wq_t = din("wq_t", [HQ, 128, NK, D], BF16)
    wk_t = din("wk_t", [128, NK, HKV * D], BF16)
    wv_t = din("wv_t", [128, NK, HKV * D], BF16)
    wo_t = din("wo_t", [DM // 512, 128, HQ, 512], BF16)
    wsq = din("wsq", [128, HQ, D], BF16)
    wsk = din("wsk", [128, HQ, D], BF16)
    cosT = din("cosT", [D, S], F32)
    sinsg = din("sinsg", [D, S], F32)
    maskbig = din("maskbig", [D, 896], BF16)
    onesf_in = din("onesf", [1, 128], mybir.dt.float16)
    out = nc.dram_tensor("out", [S, DM], F32, kind="ExternalOutput").ap()

    with SplitDrainTileContext(nc) as tc, ExitStack() as octx:
        # ---------------- persistent sbuf ----------------
        pers = octx.enter_context(tc.tile_pool(name="pers", bufs=1))
        kT_sb = pers.tile([128, HKV, S], BF16, tag="kT")         # 8KB/p
        v_sb = pers.tile([128, S // 128, HKV * D], BF16, tag="v")  # 8KB/p
        cos_sb = pers.tile([128, S], F32, tag="cos")             # 8KB/p
        sin_sb = pers.tile([128, S], F32, tag="sin")             # 8KB/p
        mask_sb = pers.tile([128, 896], BF16, tag="mask")
        wsq_sb = pers.tile([128, HQ, D], BF16, tag="wsq")        # 2KB/p
        wsk_sb = pers.tile([128, HQ, D], BF16, tag="wsk")
        ones_bf = pers.tile([128, 1], BF16, tag="ones_bf")
        ones_f = pers.tile([1, 128], mybir.dt.float16, tag="ones_f")

        nc.any.memset(ones_bf[:], 1.0)
        nc.sync.dma_start(ones_f[:], onesf_in[:, :])

        # ---------------- phase A: k/v projections ----------------
        with ExitStack() as actx:
            wkv_pool = actx.enter_context(tc.tile_pool(name="wkv", bufs=1))
            hstA_pool = actx.enter_context(tc.tile_pool(name="hstA", bufs=3))
            ptmpA = actx.enter_context(tc.tile_pool(name="ptmpA", bufs=3))
            psA = actx.enter_context(tc.tile_pool(name="psA", bufs=2, space="PSUM"))

            wk_sb = wkv_pool.tile([128, NK, HKV * D], BF16, tag="wk")  # 16KB/p
            wv_sb = wkv_pool.tile([128, NK, HKV * D], BF16, tag="wv")
            nc.sync.dma_start(wk_sb[:], wk_t[:, :, :])
            nc.sync.dma_start(wv_sb[:], wv_t[:, :, :])

            for n in range(NSL):
                sl = slice(n * SQ, (n + 1) * SQ)
                hst = hstA_pool.tile([128, NK, SQ], BF16, tag="hstA")
                nc.sync.dma_start(hst[:], hsT_t[n])
                if n == 0:
                    # issue after the critical-path loads so they don't
                    # steal startup DMA bandwidth
                    nc.sync.dma_start(cos_sb[:], cosT[:, :])
                    nc.sync.dma_start(sin_sb[:], sinsg[:, :])
                    nc.sync.dma_start(mask_sb[:], maskbig[:, :])
                    nc.sync.dma_start(wsq_sb[:], wsq[:, :, :])
                    nc.sync.dma_start(wsk_sb[:], wsk[:, :, :])
                # kT (transposed layout) + rope
                for kv in range(HKV):
                    ps = psA.tile([128, SQ], F32, tag="ps_k")
                    for k in range(NK):
                        nc.tensor.matmul(
                            ps[:], wk_sb[:, k, kv * D:(kv + 1) * D], hst[:, k, :],
                            start=(k == 0), stop=(k == NK - 1))
                    _rope(nc, ptmpA, kT_sb[:, kv, sl], ps,
                          cos_sb[:, sl], sin_sb[:, sl])
                # v natural layout [s, dv]
                for s4 in range(SQ // 128):
                    ps = psA.tile([128, HKV * D], F32, tag="ps_v")
                    for k in range(NK):
                        nc.tensor.matmul(
                            ps[:], hst[:, k, s4 * 128:(s4 + 1) * 128], wv_sb[:, k, :],
                            start=(k == 0), stop=(k == NK - 1))
                    nc.scalar.copy(v_sb[:, n * 4 + s4, :], ps[:])

        # attnT is only needed from phase B on; allocating it after phase A
        # exits lets phase A use the space for deeper hst prefetch
        attnT_pool = octx.enter_context(tc.tile_pool(name="attnTp", bufs=1))
        attnT_sb = attnT_pool.tile([128, HQ, S], BF16, tag="attnT")  # 32KB/p

        # ---------------- phase B: q proj + attention ----------------
        with ExitStack() as bctx:
            hstB_pool = bctx.enter_context(tc.tile_pool(name="hstB", bufs=1))
            wq_pool = bctx.enter_context(tc.tile_pool(name="wqp", bufs=2))
            qT_pool = bctx.enter_context(tc.tile_pool(name="qTp", bufs=2))
            exp_pool = bctx.enter_context(tc.tile_pool(name="expp", bufs=16))
            tmpB = bctx.enter_context(tc.tile_pool(name="tmpB", bufs=2))
            nrm_pool = bctx.enter_context(tc.tile_pool(name="nrmp", bufs=3))
            p_q = bctx.enter_context(tc.tile_pool(name="p_q", bufs=1, space="PSUM"))
            p_s = bctx.enter_context(tc.tile_pool(name="p_s", bufs=2, space="PSUM"))
            p_at = bctx.enter_context(tc.tile_pool(name="p_at", bufs=2, space="PSUM"))
            p_sm = bctx.enter_context(tc.tile_pool(name="p_sm", bufs=1, space="PSUM"))
            p_dl = bctx.enter_context(tc.tile_pool(name="p_dl", bufs=1, space="PSUM"))
            p_bc = bctx.enter_context(tc.tile_pool(name="p_bc", bufs=1, space="PSUM"))

            def _finish_head(st):
                """Deferred per-head epilogue: delta correction + normalize."""
                h, kv, hsl, hqT, ps_at, rc = st
                ps_dl = p_dl.tile([128, SQ], F32, tag="ps_dl")
                nc.tensor.matmul(ps_dl[:], wsq_sb[:, h, :], hqT[:, h, :],
                                 start=True, stop=False)
                nc.tensor.matmul(ps_dl[:], wsk_sb[:, h, :], kT_sb[:, kv, hsl],
                                 start=False, stop=True)
                ps_b = p_bc.tile([128, SQ], F32, tag="ps_b")
                nc.tensor.matmul(ps_b[:], ones_f[:], rc[:], start=True, stop=True)
                bc_sb = tmpB.tile([128, SQ], F32, tag="bc_sb")
                nc.vector.tensor_copy(bc_sb[:], ps_b[:])
                t1 = tmpB.tile([128, SQ], F32, tag="t1")
                nc.vector.tensor_mul(t1[:], ps_at[:], bc_sb[:])
                nc.vector.tensor_add(attnT_sb[:, h, hsl], t1[:], ps_dl[:])

            prev = None
            for n in range(NSL):
                sl = slice(n * SQ, (n + 1) * SQ)
                hst = hstB_pool.tile([128, NK, SQ], BF16, tag="hstB")
                nc.sync.dma_start(hst[:], hsT_t[n])
                qT = qT_pool.tile([128, HQ, SQ], BF16, tag="qT")
                for h in range(HQ):
                    wqc = wq_pool.tile([128, NK, D], BF16, tag="wqc")
                    nc.sync.dma_start(wqc[:], wq_t[h])
                    ps = p_q.tile([128, SQ], F32, tag="ps_q")
                    for k in range(NK):
                        nc.tensor.matmul(ps[:], wqc[:, k, :], hst[:, k, :],
                                         start=(k == 0), stop=(k == NK - 1))
                    _rope(nc, tmpB, qT[:, h, :], ps, cos_sb[:, sl], sin_sb[:, sl])

                nblk = 4 * (n + 1)
                for h in range(HQ):
                    kv = h // GROUPS
                    qh_r = qT[:, h, :]
                    ps_at = p_at.tile([128, SQ], F32, tag="ps_at")
                    ps_sm = p_sm.tile([1, SQ], F32, tag="ps_sm")
                    exs = []
                    for t in range(nblk):
                        ps_sc = p_s.tile([128, SQ], F32, tag="ps_sc")
                        nc.tensor.matmul(
                            ps_sc[:],
                            kT_sb[:, kv, t * 128:(t + 1) * 128],
                            qh_r, start=True, stop=True)
                        ex = exp_pool.tile([128, SQ], BF16, tag="ex")
                        nc.scalar.activation(ex[:], ps_sc[:],
                                             mybir.ActivationFunctionType.Exp,
                                             scale=SCALE)
                        if t >= 4 * n:
                            r = t - 4 * n
                            exm = exp_pool.tile([128, SQ], BF16, tag="exm")
                            nc.vector.tensor_mul(
                                exm[:], ex[:], mask_sb[:, 384 - 128 * r: 896 - 128 * r])
                            ex = exm
                        nc.tensor.matmul(ps_at[:], v_sb[:, t, kv * D:(kv + 1) * D],
                                         ex[:], start=(t == 0), stop=(t == nblk - 1))
                        exs.append(ex)
                    # sums as an end-burst so ps_sm frees early next head
                    for t, ex in enumerate(exs):
                        nc.tensor.matmul(ps_sm[:], ones_bf[:], ex[:],
                                         start=(t == 0), stop=(t == nblk - 1))
                    # 1/sums as exp(-ln(sums)) on ACT: faster than DVE
                    # reciprocal and keeps DVE off the critical path
                    lns = nrm_pool.tile([1, SQ], F32, tag="lns")
                    nc.scalar.activation(lns[:], ps_sm[:],
                                         mybir.ActivationFunctionType.Ln)
                    rc = nrm_pool.tile([1, SQ], mybir.dt.float16, tag="rc")
                    nc.scalar.activation(rc[:], lns[:],
                                         mybir.ActivationFunctionType.Exp,
                                         scale=-1.0)
                    # normalize the PREVIOUS head now: its reciprocal has had a
                    # full head of PE work to finish, so PE never waits on DVE
                    if prev is not None:
                        _finish_head(prev)
                    prev = (h, kv, sl, qT, ps_at, rc)
            _finish_head(prev)

        # ---------------- phase C: output projection ----------------
        with ExitStack() as cctx:
            wo_pool = cctx.enter_context(tc.tile_pool(name="wop", bufs=2))
            o_pool = cctx.enter_context(tc.tile_pool(name="op", bufs=4))
            p_o = cctx.enter_context(tc.tile_pool(name="p_o", bufs=4, space="PSUM"))
            for j in range(DM // 512):
                wo_sb = wo_pool.tile([128, HQ, 512], BF16, tag="wo")
                nc.sync.dma_start(wo_sb[:], wo_t[j])
                for m in range(S // 128):
                    ps = p_o.tile([128, 512], F32, tag="ps_o")
                    for t2 in range(HQ):
                        nc.tensor.matmul(ps[:], attnT_sb[:, t2, m * 128:(m + 1) * 128],
                                         wo_sb[:, t2, :],
                                         start=(t2 == 0), stop=(t2 == HQ - 1))
                    ot = o_pool.tile([128, 512], F32, tag="ot")
                    nc.scalar.copy(ot[:], ps[:])
                    nc.sync.dma_start(out[m * 128:(m + 1) * 128,
                                          j * 512:(j + 1) * 512], ot[:])
    _split_excess_waits(nc)
    return nc


# ---------------------------------------------------------------- host side
_CACHE = {}


def _prep_core_inputs(inputs, core):
    b, g = core // TP, core % TP
    hs = np.asarray(inputs["hidden_states"])[b]          # [S, DM] f32
    pos = np.asarray(inputs["position_ids"])[b]          # [S] int32
    Wq, Wk, Wv, Wo = (np.asarray(inputs[k]) for k in ("Wq", "Wk", "Wv", "Wo"))
    Ws_q, Ws_k = np.asarray(inputs["Ws_q"]), np.asarray(inputs["Ws_k"])

    qh0 = g * HQ                 # first global q head
    kv0 = g * HKV                # first global kv head

    inv_freq = 1.0 / (THETA ** (np.arange(0, D, 2, dtype=np.float64) / D))
    freqs = pos.astype(np.float64)[:, None] * inv_freq[None, :]   # [S, 64]
    cos = np.cos(freqs).astype(np.float32)
    sin = np.sin(freqs).astype(np.float32)
    cosT = np.ascontiguousarray(np.concatenate([cos, cos], axis=1).T)   # [128,S]
    sinsg = np.ascontiguousarray(np.concatenate([-sin, sin], axis=1).T)

    ii = np.arange(128)[:, None]
    cc = np.arange(896)[None, :]
    maskbig = ((cc - 384) >= ii).astype(BF_NP)

    # pre-tile into exact on-chip layouts (contiguous per-partition DMAs)
    hsT = hs.T.astype(BF_NP)                                   # [DM, S]
    hsT_t = np.ascontiguousarray(
        hsT.reshape(NK, 128, NSL, SQ).transpose(2, 1, 0, 3))   # [n, p, k, s]
    wq_c = Wq[:, qh0 * D:(qh0 + HQ) * D].astype(BF_NP)         # [DM, 1024]
    wq_t = np.ascontiguousarray(
        wq_c.reshape(NK, 128, HQ, D).transpose(2, 1, 0, 3))    # [h, p, k, m]
    wk_c = Wk[:, kv0 * D:(kv0 + HKV) * D].astype(BF_NP)
    wk_t = np.ascontiguousarray(
        wk_c.reshape(NK, 128, HKV * D).transpose(1, 0, 2))     # [p, k, m]
    wv_c = Wv[:, kv0 * D:(kv0 + HKV) * D].astype(BF_NP)
    wv_t = np.ascontiguousarray(
        wv_c.reshape(NK, 128, HKV * D).transpose(1, 0, 2))
    wo_c = Wo[qh0 * D:(qh0 + HQ) * D, :].astype(BF_NP)         # [1024, DM]
    wo_t = np.ascontiguousarray(
        wo_c.reshape(HQ, 128, DM // 512, 512).transpose(2, 1, 0, 3))  # [j,p,t,m]
    wsq_t = np.ascontiguousarray(
        Ws_q[qh0:qh0 + HQ].transpose(1, 0, 2)).astype(np.float32)  # [d, h, e]
    wsk_t = np.ascontiguousarray(
        Ws_k[qh0:qh0 + HQ].transpose(1, 0, 2)).astype(np.float32)
    return {
        "hsT_t": hsT_t,
        "wq_t": wq_t,
        "wk_t": wk_t,
        "wv_t": wv_t,
        "wo_t": wo_t,
        "wsq": wsq_t.astype(BF_NP),
        "wsk": wsk_t.astype(BF_NP),
        "cosT": cosT,
        "sinsg": sinsg,
        "maskbig": maskbig,
        "onesf": np.ones((1, 128), dtype=np.float16),
    }


def run(inputs, trace=False):
    if "nc" not in _CACHE:
        _CACHE["nc"] = build_kernel()
    nc = _CACHE["nc"]
    in_maps = [_prep_core_inputs(inputs, c) for c in range(N_CORES)]
    res = bass_utils.run_bass_kernel_spmd(
        nc, in_maps, core_ids=list(range(N_CORES)), trace=trace)
    full = np.zeros((B, S, DM), dtype=np.float32)
    for c in range(N_CORES):
        full[c // TP] += res.results[c]["out"]
    return full, res


def kernel(**inputs) -> np.ndarray:
    full, _ = run(inputs, trace=False)
    return full


# revision 58
# speedup vs baseline: 1.2552x; 1.0219x over previous
"""Trainium2 Bass kernel for nn_AttnApproximator (GQA attention + RoPE +
per-head shift correction), sharded over 8 NeuronCores.

Sharding: tensor-parallel over heads (4 groups of 8 query heads / 2 KV
heads) x data-parallel over batch (B=2) -> 8 cores. Each core computes a
partial output contribution [S, Dm] (its heads' slice of the attn @ Wo
contraction); the host sums the 4 head-group partials per batch element.

Per-core pipeline (everything stays transposed so no on-chip transposes
are needed):
  phase A: kT = (hs @ Wk).T with RoPE, v = hs @ Wv          (per s-slice)
  phase B: per s-slice of 512: qT = (hs @ Wq).T with RoPE, then causal
           attention per head in scores-transposed form:
             scoresT[sk, sq] = kT_tile.T @ qT    (f32r matmuls)
             expT = exp(scale * scoresT)          (ACT, bf16 out)
             attnT[dv, sq] += v_tile.T @ expT     (bf16 matmuls, PSUM acc)
             sums[1, sq]  += ones.T @ expT
           then delta = Ws_q.T-style per-head correction, normalize by
           broadcast(1/sums) via a K=1 PE matmul, add delta -> attnT(bf16)
  phase C: out[s, dm] = attnT.T @ Wo              (bf16 matmuls)
"""

import math
import numpy as np
import ml_dtypes

import bass_rust
import concourse.bass as bass
import concourse.tile as tile
from concourse import mybir
from concourse import bass_utils
from concourse.vector_clock import ScopedClock
from contextlib import ExitStack

# ---------------------------------------------------------------- constants
B, S, DM = 2, 2048, 4096
H, KV, D = 32, 8, 128
N_CORES = 8
TP = 4                    # head groups
HQ = H // TP              # 8 q heads per core
HKV = KV // TP            # 2 kv heads per core
GROUPS = H // KV          # 4
THETA = 10000.0
SQ = 512                  # s-slice width
NSL = S // SQ             # 4 slices
NK = DM // 128            # 32 contraction tiles
SCALE = 1.0 / math.sqrt(D)

F32 = mybir.dt.float32
F32R = mybir.dt.float32r
BF16 = mybir.dt.bfloat16
BF_NP = ml_dtypes.bfloat16


# ------------------------------------------------- walrus drain-wait fixup
class SplitDrainTileContext(tile.TileContext):
    """This container's walrus rejects >1 sync wait on the SP tail-drain
    CTRL instruction; split the gathered waits onto chained SP nops."""

    MAX_WAITS = 1

    def _drain_and_barrier(self, tick_clock, wait_clock):
        nc = self.nc
        drain_inst = nc.sync.drain()
        wait_clock.add_sem_waits(
            drain_inst.ins, ScopedClock({None: tick_clock.global_clock})
        )
        si = drain_inst.ins.sync_info
        waits = list(si.on_wait) if si is not None else []
        mw = self.MAX_WAITS
        if len(waits) > mw:
            drain_inst.ins.sync_info = bass_rust.SyncInfo(
                on_wait=waits[:mw], on_update=list(si.on_update)
            )
            for k in range(mw, len(waits), mw):
                nop = nc.sync.nop(nofuse=True, hint="drain_wait_split")
                nop.ins.sync_info = bass_rust.SyncInfo(
                    on_wait=waits[k : k + mw], on_update=[]
                )
        nc.all_engine_barrier()
        assert self.sems is not None
        popped = nc._tile_sem_poison_stack.pop()
        assert popped is self._sem_poison
        nc.clear_and_free_semaphores(list(self.sems.allocated().values()))
        nc.all_engine_barrier()


def _split_excess_waits(nc):
    """This walrus accepts 1 sync wait per instruction (2 for
    EventSemaphore). Tile emits more; move the excess onto same-engine
    NoOp carriers inserted immediately before the over-limit instruction."""
    uid = 0
    for fn in nc.m.functions:
        for bb in fn.blocks:
            new, changed = [], False
            for inst in bb.instructions:
                si = inst.sync_info
                waits = list(si.on_wait) if si is not None else []
                cap = 2 if inst.opcode == "EventSemaphore" else 1
                if len(waits) > cap:
                    changed = True
                    for w in waits[:-cap]:
                        nop = mybir.InstNoOp(
                            name=f"I-wsplit-{uid}",
                            engine=inst.engine,
                            bass_nofuse=True,
                            sync_info=mybir.SyncInfo(on_wait=[w], on_update=[]),
                        )
                        uid += 1
                        new.append(nop)
                    inst.sync_info = bass_rust.SyncInfo(
                        on_wait=waits[-cap:], on_update=list(si.on_update))
                new.append(inst)
            if changed:
                bb.instructions = new


# ---------------------------------------------------------------- builder
def _rope(nc, tmp_pool, out_ap, in_ps, cos_sl, sin_sl):
    """out = in*cosT + swap_halves(in)*sinT_signed ; in_ps is PSUM f32.
    First op drains PSUM via ACT so the bank frees fast (p_q bufs=1)."""
    q_sb = tmp_pool.tile([128, SQ], F32, tag="rope_q")
    nc.scalar.copy(q_sb[:], in_ps[:])
    sw = tmp_pool.tile([128, SQ], F32, tag="rope_sw")
    nc.vector.tensor_copy(sw[0:64, :], q_sb[64:128, :])
    nc.vector.tensor_copy(sw[64:128, :], q_sb[0:64, :])
    nc.vector.tensor_mul(sw[:], sw[:], sin_sl)
    t2 = tmp_pool.tile([128, SQ], F32, tag="rope_t2")
    nc.vector.tensor_mul(t2[:], q_sb[:], cos_sl)
    nc.vector.tensor_add(out_ap, t2[:], sw[:])


def build_kernel():
    nc = bass.Bass("TRN2", target_bir_lowering=False, debug=False,
                   num_devices=N_CORES)

    # All inputs are pre-tiled on the host into the exact sbuf layouts so
    # every DMA is contiguous per partition (few large descriptors).
    din = lambda n, shp, dt: nc.dram_tensor(n, shp, dt, kind="ExternalInput").ap()
    hsT_t = din("hsT_t", [NSL, 128, NK, SQ], BF16)
    wq_t = din("wq_t", [HQ, 128, NK, D], BF16)
    wk_t = din("wk_t", [128, NK, HKV * D], BF16)
    wv_t = din("wv_t", [128, NK, HKV * D], BF16)
    wo_t = din("wo_t", [DM // 512, 128, HQ, 512], BF16)
    wsq = din("wsq", [128, HQ, D], BF16)
    wsk = din("wsk", [128, HQ, D], BF16)
    cosT = din("cosT", [D, S], F32)
    sinsg = din("sinsg", [D, S], F32)
    maskbig = din("maskbig", [D, 896], BF16)
    onesf_in = din("onesf", [1, 128], mybir.dt.float16)
    out = nc.dram_tensor("out", [S, DM], F32, kind="ExternalOutput").ap()

    with SplitDrainTileContext(nc) as tc, ExitStack() as octx:
        # ---------------- persistent sbuf ----------------
        pers = octx.enter_context(tc.tile_pool(name="pers", bufs=1))
        kT_sb = pers.tile([128, HKV, S], BF16, tag="kT")         # 8KB/p
        v_sb = pers.tile([128, S // 128, HKV * D], BF16, tag="v")  # 8KB/p
        cos_sb = pers.tile([128, S], F32, tag="cos")             # 8KB/p
        sin_sb = pers.tile([128, S], F32, tag="sin")             # 8KB/p
        mask_sb = pers.tile([128, 896], BF16, tag="mask")
        wsq_sb = pers.tile([128, HQ, D], BF16, tag="wsq")        # 2KB/p
        wsk_sb = pers.tile([128, HQ, D], BF16, tag="wsk")
        ones_bf = pers.tile([128, 1], BF16, tag="ones_bf")
        ones_f = pers.tile([1, 128], mybir.dt.float16, tag="ones_f")
        qT0 = pers.tile([128, HQ, SQ], BF16, tag="qT0")          # 8KB/p
        wq_pool = octx.enter_context(tc.tile_pool(name="wqp", bufs=2))

        nc.any.memset(ones_bf[:], 1.0)
        nc.sync.dma_start(ones_f[:], onesf_in[:, :])

        # ---------------- phase A: k/v projections ----------------
        with ExitStack() as actx:
            wkv_pool = actx.enter_context(tc.tile_pool(name="wkv", bufs=1))
            hstA_pool = actx.enter_context(tc.tile_pool(name="hstA", bufs=2))
            ptmpA = actx.enter_context(tc.tile_pool(name="ptmpA", bufs=3))
            psA = actx.enter_context(tc.tile_pool(name="psA", bufs=2, space="PSUM"))

            wk_sb = wkv_pool.tile([128, NK, HKV * D], BF16, tag="wk")  # 16KB/p
            wv_sb = wkv_pool.tile([128, NK, HKV * D], BF16, tag="wv")
            # split loads so the first kv-proj chain starts sooner
            nc.sync.dma_start(wk_sb[:, :, 0:D], wk_t[:, :, 0:D])
            nc.sync.dma_start(wk_sb[:, :, D:HKV * D], wk_t[:, :, D:HKV * D])
            nc.sync.dma_start(wv_sb[:], wv_t[:, :, :])

            for n in range(NSL):
                sl = slice(n * SQ, (n + 1) * SQ)
                hst = hstA_pool.tile([128, NK, SQ], BF16, tag="hstA")
                nc.sync.dma_start(hst[:], hsT_t[n])
                if n == 0:
                    # issue after the critical-path loads so they don't
                    # steal startup DMA bandwidth
                    nc.sync.dma_start(cos_sb[:], cosT[:, :])
                    nc.sync.dma_start(sin_sb[:], sinsg[:, :])
                    nc.sync.dma_start(mask_sb[:], maskbig[:, :])
                    nc.sync.dma_start(wsq_sb[:], wsq[:, :, :])
                    nc.sync.dma_start(wsk_sb[:], wsk[:, :, :])
                # kT (transposed layout) + rope
                for kv in range(HKV):
                    ps = psA.tile([128, SQ], F32, tag="ps_k")
                    for k in range(NK):
                        nc.tensor.matmul(
                            ps[:], wk_sb[:, k, kv * D:(kv + 1) * D], hst[:, k, :],
                            start=(k == 0), stop=(k == NK - 1))
                    _rope(nc, ptmpA, kT_sb[:, kv, sl], ps,
                          cos_sb[:, sl], sin_sb[:, sl])
                # v natural layout [s, dv]
                for s4 in range(SQ // 128):
                    ps = psA.tile([128, HKV * D], F32, tag="ps_v")
                    for k in range(NK):
                        nc.tensor.matmul(
                            ps[:], hst[:, k, s4 * 128:(s4 + 1) * 128], wv_sb[:, k, :],
                            start=(k == 0), stop=(k == NK - 1))
                    nc.scalar.copy(v_sb[:, n * 4 + s4, :], ps[:])
                if n == 0:
                    # fuse slice-0 q-projection here so phase B can start
                    # attention immediately (its own loads reuse phase-A
                    # address space and only land after phase A drains)
                    for h in range(HQ):
                        wqc = wq_pool.tile([128, NK, D], BF16, tag="wqc")
                        nc.sync.dma_start(wqc[:], wq_t[h])
                        ps = psA.tile([128, SQ], F32, tag="ps_q0")
                        for k in range(NK):
                            nc.tensor.matmul(ps[:], wqc[:, k, :], hst[:, k, :],
                                             start=(k == 0), stop=(k == NK - 1))
                        _rope(nc, ptmpA, qT0[:, h, :], ps,
                              cos_sb[:, 0:SQ], sin_sb[:, 0:SQ])

        # attnT is only needed from phase B on; allocating it after phase A
        # exits lets phase A use the space for deeper hst prefetch
        attnT_pool = octx.enter_context(tc.tile_pool(name="attnTp", bufs=1))
        attnT_sb = attnT_pool.tile([128, HQ, S], BF16, tag="attnT")  # 32KB/p

        # ---------------- phase B: q proj + attention ----------------
        with ExitStack() as bctx:
            hstB_pool = bctx.enter_context(tc.tile_pool(name="hstB", bufs=1))
            qT_pool = bctx.enter_context(tc.tile_pool(name="qTp", bufs=2))
            exp_pool = bctx.enter_context(tc.tile_pool(name="expp", bufs=16))
            tmpB = bctx.enter_context(tc.tile_pool(name="tmpB", bufs=2))
            nrm_pool = bctx.enter_context(tc.tile_pool(name="nrmp", bufs=3))
            p_q = bctx.enter_context(tc.tile_pool(name="p_q", bufs=1, space="PSUM"))
            p_s = bctx.enter_context(tc.tile_pool(name="p_s", bufs=2, space="PSUM"))
            p_at = bctx.enter_context(tc.tile_pool(name="p_at", bufs=2, space="PSUM"))
            p_sm = bctx.enter_context(tc.tile_pool(name="p_sm", bufs=1, space="PSUM"))
            p_dl = bctx.enter_context(tc.tile_pool(name="p_dl", bufs=1, space="PSUM"))
            p_bc = bctx.enter_context(tc.tile_pool(name="p_bc", bufs=1, space="PSUM"))

            def _finish_head(st):
                """Deferred per-head epilogue: delta correction + normalize."""
                h, kv, hsl, hqT, ps_at, rc = st
                ps_dl = p_dl.tile([128, SQ], F32, tag="ps_dl")
                nc.tensor.matmul(ps_dl[:], wsq_sb[:, h, :], hqT[:, h, :],
                                 start=True, stop=False)
                nc.tensor.matmul(ps_dl[:], wsk_sb[:, h, :], kT_sb[:, kv, hsl],
                                 start=False, stop=True)
                ps_b = p_bc.tile([128, SQ], F32, tag="ps_b")
                nc.tensor.matmul(ps_b[:], ones_f[:], rc[:], start=True, stop=True)
                bc_sb = tmpB.tile([128, SQ], F32, tag="bc_sb")
                nc.vector.tensor_copy(bc_sb[:], ps_b[:])
                t1 = tmpB.tile([128, SQ], F32, tag="t1")
                nc.vector.tensor_mul(t1[:], ps_at[:], bc_sb[:])
                nc.vector.tensor_add(attnT_sb[:, h, hsl], t1[:], ps_dl[:])

            prev = None
            for n in range(NSL):
                sl = slice(n * SQ, (n + 1) * SQ)
                if n == 0:
                    qT = qT0
                else:
                    hst = hstB_pool.tile([128, NK, SQ], BF16, tag="hstB")
                    nc.sync.dma_start(hst[:], hsT_t[n])
                    qT = qT_pool.tile([128, HQ, SQ], BF16, tag="qT")
                    for h in range(HQ):
                        wqc = wq_pool.tile([128, NK, D], BF16, tag="wqc")
                        nc.sync.dma_start(wqc[:], wq_t[h])
                        ps = p_q.tile([128, SQ], F32, tag="ps_q")
                        for k in range(NK):
                            nc.tensor.matmul(ps[:], wqc[:, k, :], hst[:, k, :],
                                             start=(k == 0), stop=(k == NK - 1))
                        _rope(nc, tmpB, qT[:, h, :], ps,
                              cos_sb[:, sl], sin_sb[:, sl])

                nblk = 4 * (n + 1)
                for h in range(HQ):
                    kv = h // GROUPS
                    qh_r = qT[:, h, :]
                    ps_at = p_at.tile([128, SQ], F32, tag="ps_at")
                    ps_sm = p_sm.tile([1, SQ], F32, tag="ps_sm")
                    exs = []
                    for t in range(nblk):
                        ps_sc = p_s.tile([128, SQ], F32, tag="ps_sc")
                        nc.tensor.matmul(
                            ps_sc[:],
                            kT_sb[:, kv, t * 128:(t + 1) * 128],
                            qh_r, start=True, stop=True)
                        ex = exp_pool.tile([128, SQ], BF16, tag="ex")
                        nc.scalar.activation(ex[:], ps_sc[:],
                                             mybir.ActivationFunctionType.Exp,
                                             scale=SCALE)
                        if t >= 4 * n:
                            r = t - 4 * n
                            exm = exp_pool.tile([128, SQ], BF16, tag="exm")
                            nc.vector.tensor_mul(
                                exm[:], ex[:], mask_sb[:, 384 - 128 * r: 896 - 128 * r])
                            ex = exm
                        nc.tensor.matmul(ps_at[:], v_sb[:, t, kv * D:(kv + 1) * D],
                                         ex[:], start=(t == 0), stop=(t == nblk - 1))
                        exs.append(ex)
                    # sums as an end-burst so ps_sm frees early next head
                    for t, ex in enumerate(exs):
                        nc.tensor.matmul(ps_sm[:], ones_bf[:], ex[:],
                                         start=(t == 0), stop=(t == nblk - 1))
                    # 1/sums as exp(-ln(sums)) on ACT: faster than DVE
                    # reciprocal and keeps DVE off the critical path
                    lns = nrm_pool.tile([1, SQ], F32, tag="lns")
                    nc.scalar.activation(lns[:], ps_sm[:],
                                         mybir.ActivationFunctionType.Ln)
                    rc = nrm_pool.tile([1, SQ], mybir.dt.float16, tag="rc")
                    nc.scalar.activation(rc[:], lns[:],
                                         mybir.ActivationFunctionType.Exp,
                                         scale=-1.0)
                    # normalize the PREVIOUS head now: its reciprocal has had a
                    # full head of PE work to finish, so PE never waits on DVE
                    if prev is not None:
                        _finish_head(prev)
                    prev = (h, kv, sl, qT, ps_at, rc)
            _finish_head(prev)

        # ---------------- phase C: output projection ----------------
        with ExitStack() as cctx:
            wo_pool = cctx.enter_context(tc.tile_pool(name="wop", bufs=2))
            o_pool = cctx.enter_context(tc.tile_pool(name="op", bufs=4))
            p_o = cctx.enter_context(tc.tile_pool(name="p_o", bufs=4, space="PSUM"))
            for j in range(DM // 512):
                wo_sb = wo_pool.tile([128, HQ, 512], BF16, tag="wo")
                nc.sync.dma_start(wo_sb[:], wo_t[j])
                for m in range(S // 128):
                    ps = p_o.tile([128, 512], F32, tag="ps_o")
                    for t2 in range(HQ):
                        nc.tensor.matmul(ps[:], attnT_sb[:, t2, m * 128:(m + 1) * 128],
                                         wo_sb[:, t2, :],
                                         start=(t2 == 0), stop=(t2 == HQ - 1))
                    ot = o_pool.tile([128, 512], F32, tag="ot")
                    nc.scalar.copy(ot[:], ps[:])
                    nc.sync.dma_start(out[m * 128:(m + 1) * 128,
                                          j * 512:(j + 1) * 512], ot[:])
    _split_excess_waits(nc)
    return nc


# ---------------------------------------------------------------- host side
_CACHE = {}


def _prep_core_inputs(inputs, core):
    b, g = core // TP, core % TP
    hs = np.asarray(inputs["hidden_states"])[b]          # [S, DM] f32
    pos = np.asarray(inputs["position_ids"])[b]          # [S] int32
    Wq, Wk, Wv, Wo = (np.asarray(inputs[k]) for k in ("Wq", "Wk", "Wv", "Wo"))
    Ws_q, Ws_k = np.asarray(inputs["Ws_q"]), np.asarray(inputs["Ws_k"])

    qh0 = g * HQ                 # first global q head
    kv0 = g * HKV                # first global kv head

    inv_freq = 1.0 / (THETA ** (np.arange(0, D, 2, dtype=np.float64) / D))
    freqs = pos.astype(np.float64)[:, None] * inv_freq[None, :]   # [S, 64]
    cos = np.cos(freqs).astype(np.float32)
    sin = np.sin(freqs).astype(np.float32)
    cosT = np.ascontiguousarray(np.concatenate([cos, cos], axis=1).T)   # [128,S]
    sinsg = np.ascontiguousarray(np.concatenate([-sin, sin], axis=1).T)

    ii = np.arange(128)[:, None]
    cc = np.arange(896)[None, :]
    maskbig = ((cc - 384) >= ii).astype(BF_NP)

    # pre-tile into exact on-chip layouts (contiguous per-partition DMAs)
    hsT = hs.T.astype(BF_NP)                                   # [DM, S]
    hsT_t = np.ascontiguousarray(
        hsT.reshape(NK, 128, NSL, SQ).transpose(2, 1, 0, 3))   # [n, p, k, s]
    wq_c = Wq[:, qh0 * D:(qh0 + HQ) * D].astype(BF_NP)         # [DM, 1024]
    wq_t = np.ascontiguousarray(
        wq_c.reshape(NK, 128, HQ, D).transpose(2, 1, 0, 3))    # [h, p, k, m]
    wk_c = Wk[:, kv0 * D:(kv0 + HKV) * D].astype(BF_NP)
    wk_t = np.ascontiguousarray(
        wk_c.reshape(NK, 128, HKV * D).transpose(1, 0, 2))     # [p, k, m]
    wv_c = Wv[:, kv0 * D:(kv0 + HKV) * D].astype(BF_NP)
    wv_t = np.ascontiguousarray(
        wv_c.reshape(NK, 128, HKV * D).transpose(1, 0, 2))
    wo_c = Wo[qh0 * D:(qh0 + HQ) * D, :].astype(BF_NP)         # [1024, DM]
    wo_t = np.ascontiguousarray(
        wo_c.reshape(HQ, 128, DM // 512, 512).transpose(2, 1, 0, 3))  # [j,p,t,m]
    wsq_t = np.ascontiguousarray(
        Ws_q[qh0:qh0 + HQ].transpose(1, 0, 2)).astype(np.float32)  # [d, h, e]
    wsk_t = np.ascontiguousarray(
        Ws_k[qh0:qh0 + HQ].transpose(1, 0, 2)).astype(np.float32)
    return {
        "hsT_t": hsT_t,
        "wq_t": wq_t,
        "wk_t": wk_t,
        "wv_t": wv_t,
        "wo_t": wo_t,
        "wsq": wsq_t.astype(BF_NP),
        "wsk": wsk_t.astype(BF_NP),
        "cosT": cosT,
        "sinsg": sinsg,
        "maskbig": maskbig,
        "onesf": np.ones((1, 128), dtype=np.float16),
    }


def run(inputs, trace=False):
    if "nc" not in _CACHE:
        _CACHE["nc"] = build_kernel()
    nc = _CACHE["nc"]
    in_maps = [_prep_core_inputs(inputs, c) for c in range(N_CORES)]
    res = bass_utils.run_bass_kernel_spmd(
        nc, in_maps, core_ids=list(range(N_CORES)), trace=trace)
    full = np.zeros((B, S, DM), dtype=np.float32)
    for c in range(N_CORES):
        full[c // TP] += res.results[c]["out"]
    return full, res


def kernel(**inputs) -> np.ndarray:
    full, _ = run(inputs, trace=False)
    return full
